# revision 21
# baseline (speedup 1.0000x reference)
"""GQA attention layer (B=1, S=2048, D=4096, H=32, KVH=8, HD=128) on 8 TRN2
NeuronCores, tensor-parallel over heads.

Each core computes 4 query heads + their shared kv head end-to-end:
QKV projection -> RoPE -> causal attention (no-max-sub softmax, scores are
tiny) -> its slice of the wo projection. The 8 partial [S, D] outputs are
summed on the host (the "all-reduce after wo" of the sharding hint).

Device layouts (everything bf16 into the PE, fp32 PSUM accumulation):
  QT/KT  [HD=128(part), S]    from  lhsT=w[d,:], rhs=xT[d, s-tile]
  V      [S(part), HD]        via PE-transpose of VT
  scoresT[k(part), q]         lhsT=KT chunk, rhs=QT tile
  E = exp(scoresT/128) bf16; causal diagonal via 0/1 mask multiply
  attnT  [HD(part), q]        lhsT=V chunk, rhs=E  (accumulated over k)
  denom  [1, q]               lhsT=ones[128,1], rhs=E (accumulated over k)
  attnT_norm = attnT * bcast(1/denom)   (PE outer-product broadcast)
  out    [s(part), n]         lhsT=attnT_norm chunk, rhs=woT
"""

import json
import math

import ml_dtypes
import numpy as np

import concourse.bass as bass
import concourse.tile as tile
from concourse import mybir
from concourse.bass_utils import run_bass_kernel_spmd

BF16 = mybir.dt.bfloat16
F32 = mybir.dt.float32
NPBF16 = ml_dtypes.bfloat16

# Full problem constants
B, S, D = 1, 2048, 4096
H, KVH = 32, 8
HD = 128
NCORES = 8
HQ = H // NCORES  # query heads per core
MULT = 1.0
ROPE_BASE = 10000.0
ST = 512  # s-tile (PSUM bank width in fp32)


def attn_scale(seq_len=S, d_head=HD, mult=MULT):
    alpha = 1.0 / (1.0 + 4.0 * d_head / mult**2)
    lower = (math.log(seq_len) / seq_len) ** 0.5
    interp = math.exp((1.0 - alpha) * math.log(lower))
    return 1.0 / interp


def _legalize_single_wait(nc):
    """The walrus build in this container accepts only ONE sync wait per
    instruction ("Too many sync wait commands" in setupSyncWait). Split
    extra waits into preceding single-wait Drains (lowered to CTRL NOPs)
    on the same engine — same in-order stall semantics."""
    bir = json.loads(nc.to_json_bytes())
    ctr = 0
    for fn in bir["functions"]:
        for blk in fn["blocks"]:
            out = []
            for inst in blk["instructions"]:
                si = inst.get("sync_info")
                waits = (si or {}).get("on_wait") or []
                if len(waits) > 1:
                    for w in waits[:-1]:
                        ctr += 1
                        out.append(
                            {
                                "debug": inst.get("debug", 0),
                                "engine": inst["engine"],
                                "ins": [],
                                "name": f"{inst['name']}-mw{ctr}",
                                "opcode": "Drain",
                                "outs": [],
                                "sync_info": {"on_update": [], "on_wait": [w]},
                            }
                        )
                    si["on_wait"] = [waits[-1]]
                out.append(inst)
            blk["instructions"] = out
    fixed = json.dumps(bir).encode()
    nc.to_json_bytes = lambda: fixed
    return nc


def build_core_kernel(s=S, d=D, hq=HQ):
    """Bass module for one core: hq query heads + 1 kv head."""
    nst = s // ST  # s-tiles of 512
    ndk = d // 128  # contraction chunks
    nh = hq + 2  # q heads + k + v
    nnt = d // ST  # output n-tiles

    nc = bass.Bass()
    xT_d = nc.dram_tensor("xT", [d, s], BF16, kind="ExternalInput")
    wqkvT_d = nc.dram_tensor("wqkvT", [d, nh * 128], BF16, kind="ExternalInput")
    woT_d = nc.dram_tensor("woT", [hq * 128, d], BF16, kind="ExternalInput")
    cosF_d = nc.dram_tensor("cosF", [128, s], BF16, kind="ExternalInput")
    sinSg_d = nc.dram_tensor("sinSg", [128, s], BF16, kind="ExternalInput")
    maskT_d = nc.dram_tensor("maskT", [128, 128], BF16, kind="ExternalInput")
    ident_d = nc.dram_tensor("ident", [128, 128], BF16, kind="ExternalInput")
    onesc_d = nc.dram_tensor("onesc", [128, 1], BF16, kind="ExternalInput")
    onesr_d = nc.dram_tensor("onesr", [128, 128], BF16, kind="ExternalInput")
    outp_d = nc.dram_tensor("outp", [s, d], BF16, kind="ExternalOutput")

    with tile.TileContext(nc) as tc:
        with (
            tc.tile_pool(name="const", bufs=1) as cp,
            tc.tile_pool(name="qkvsb", bufs=1) as qp,
            tc.tile_pool(name="xp", bufs=3) as xp,
            tc.tile_pool(name="rp", bufs=2) as rp,
            tc.tile_pool(name="vp", bufs=2) as vp,
            tc.tile_pool(name="ep", bufs=12) as ep,
            tc.tile_pool(name="sp", bufs=2) as sp,
            tc.tile_pool(name="op", bufs=6) as op,
            tc.tile_pool(name="at", bufs=8) as atp,
        ):
            # ---- resident constants ----
            # per-chunk weight tiles so the first matmul starts after the
            # first small DMA, not after the whole 10MB weight load
            nwg = ndk // 8  # weight groups of 8 contraction chunks
            wsb4 = [
                cp.tile([128, 8, nh * 128], BF16, tag=f"w{g}", name=f"w{g}")
                for g in range(nwg)
            ]
            wqkv_r = wqkvT_d.rearrange("(g c p) n -> g p c n", c=8, p=128)
            for g in range(nwg):
                nc.gpsimd.dma_start(wsb4[g], wqkv_r[g])
            wsb = [wsb4[dk // 8][:, dk % 8, :] for dk in range(ndk)]
            cossb = cp.tile([128, s], BF16)
            nc.gpsimd.dma_start(cossb, cosF_d[:])
            sinsb = cp.tile([128, s], BF16)
            nc.gpsimd.dma_start(sinsb, sinSg_d[:])
            masksb = cp.tile([128, 128], BF16)
            nc.gpsimd.dma_start(masksb, maskT_d[:])
            identsb = cp.tile([128, 128], BF16)
            nc.gpsimd.dma_start(identsb, ident_d[:])
            onescsb = cp.tile([128, 1], BF16)
            nc.gpsimd.dma_start(onescsb, onesc_d[:])
            onescbsb = cp.tile([128, 128], BF16)
            nc.gpsimd.dma_start(onescbsb, onesr_d[:])
            wosb = [
                cp.tile([128, d], BF16, tag=f"wo{mh}", name=f"wo{mh}")
                for mh in range(hq)
            ]
            for mh in range(hq):
                nc.gpsimd.dma_start(wosb[mh], woT_d[mh * 128 : (mh + 1) * 128, :])

            # ---- persistent activations (bf16) ----
            qt_sb = [
                qp.tile([128, s], BF16, tag=f"QT{h}", name=f"QT{h}")
                for h in range(hq)
            ]
            kt_sb = qp.tile([128, s], BF16, tag="KT")
            v_sb = qp.tile([128, s], BF16, tag="V")  # [s%128 part, (s//128)*HD]

            # ================= phase A: QKV projection + RoPE =================
            with (
                tc.tile_pool(name="psA", bufs=7, space="PSUM") as psA,
                tc.tile_pool(name="psT", bufs=1, space="PSUM") as psT,
            ):
                nq = 4  # quarters per s-tile
                ndkq = ndk // nq  # contraction chunks per quarter
                for st in range(nst):
                    ssl = slice(st * ST, (st + 1) * ST)
                    acc = [
                        psA.tile([128, ST], F32, tag="acc", name=f"acc{h}")
                        for h in range(nh)
                    ]
                    # heads-major over resident xT quarters: at the next s-tile
                    # boundary only acc[0] must be free for PE to proceed
                    for quar in range(nq):
                        xta = xp.tile([128, ndkq, ST], BF16, tag="xT")
                        for dk in range(ndkq):
                            nc.sync.dma_start(
                                xta[:, dk, :],
                                xT_d[
                                    (quar * ndkq + dk) * 128 : (quar * ndkq + dk + 1)
                                    * 128,
                                    ssl,
                                ],
                            )
                        for h in range(nh):
                            for dk in range(ndkq):
                                nc.tensor.matmul(
                                    acc[h],
                                    wsb[quar * ndkq + dk][:, h * 128 : (h + 1) * 128],
                                    xta[:, dk, :],
                                    start=(quar == 0 and dk == 0),
                                    stop=(quar == nq - 1 and dk == ndkq - 1),
                                )
                    # RoPE for q heads and k; write bf16
                    for h in range(hq + 1):
                        dst = qt_sb[h] if h < hq else kt_sb
                        t1 = rp.tile([128, ST], F32, tag="t1")
                        nc.vector.tensor_mul(t1, acc[h], cossb[:, ssl])
                        tsw = rp.tile([128, ST], F32, tag="tsw")
                        nc.scalar.copy(tsw[0:64, :], acc[h][64:128, :])
                        nc.scalar.copy(tsw[64:128, :], acc[h][0:64, :])
                        nc.vector.tensor_mul(tsw, tsw, sinsb[:, ssl])
                        nc.vector.tensor_add(dst[:, ssl], t1, tsw)
                    # V: transpose [HD, s-tile] -> [s-chunk, HD] blocks
                    for j in range(ST // 128):
                        vtmp = vp.tile([128, 128], BF16, tag="vtmp")
                        nc.scalar.copy(vtmp, acc[hq + 1][:, j * 128 : (j + 1) * 128])
                        tp_ps = psT.tile([128, 128], BF16, tag="tp")
                        nc.tensor.transpose(tp_ps, vtmp, identsb)
                        sc = st * (ST // 128) + j
                        nc.vector.tensor_copy(
                            v_sb[:, sc * 128 : (sc + 1) * 128], tp_ps
                        )

            # ============ phase B: attention + output projection ============
            with (
                tc.tile_pool(name="psS", bufs=3, space="PSUM") as psS,
                tc.tile_pool(name="psD", bufs=1, space="PSUM") as psD,
                tc.tile_pool(name="psAt", bufs=4, space="PSUM") as psAt,
            ):

                def emit_wo(qt, attn_tiles):
                    # wo for the s-chunks of q-tile qt (emitted one q-tile
                    # late so the normalize tail overlaps the next q-tile's
                    # attention matmuls)
                    with nc.named_scope(f"wo{qt}"):
                        for j in range(ST // 128):
                            sc = qt * (ST // 128) + j
                            for nt in range(nnt):
                                o_ps = psS.tile(
                                    [128, ST], F32, tag="sc",
                                    name=f"wo{qt}_{j}_{nt}",
                                )
                                for mh in range(hq):
                                    nc.tensor.matmul(
                                        o_ps,
                                        attn_tiles[mh][:, j * 128 : (j + 1) * 128],
                                        wosb[mh][:, nt * ST : (nt + 1) * ST],
                                        start=(mh == 0),
                                        stop=(mh == hq - 1),
                                    )
                                osb = op.tile([128, ST], BF16, tag="osb")
                                if (j + nt) % 2 == 0:
                                    nc.vector.tensor_copy(osb, o_ps)
                                else:
                                    nc.scalar.copy(osb, o_ps)
                                nc.sync.dma_start(
                                    outp_d[
                                        sc * 128 : (sc + 1) * 128,
                                        nt * ST : (nt + 1) * ST,
                                    ],
                                    osb,
                                )

                prev_wo = None
                for qt in range(nst):
                    nk = (qt + 1) * (ST // 128)  # causal: k chunks this q-tile
                    attn_tiles = {}
                    with nc.named_scope(f"attn{qt}"):
                        # one denominator bank per q-tile: head h accumulates
                        # into partition row 32*h (distinct col-groups)
                        den4 = psD.tile([128, ST], F32, tag="den")
                        nc.vector.memset(den4, 1.0)
                        at_tiles = {
                            h: psAt.tile([128, ST], F32, tag="at", name=f"at{qt}_{h}")
                            for h in range(hq)
                        }
                        for c in range(nk):
                            # diagonal chunks: only columns >= 128*r valid
                            r = c - (nk - 4)
                            off = 128 * r if r > 0 else 0
                            w = ST - off
                            e_ts = {}
                            for h in range(hq):
                                sc_ps = psS.tile(
                                    [128, ST], F32, tag="sc", name=f"sc{qt}_{c}_{h}"
                                )
                                nc.tensor.matmul(
                                    sc_ps[:, 0:w],
                                    kt_sb[:, c * 128 : (c + 1) * 128],
                                    qt_sb[h][:, qt * ST + off : (qt + 1) * ST],
                                    start=True,
                                    stop=True,
                                )
                                e_t = ep.tile(
                                    [128, ST], BF16, tag="E", name=f"e{qt}_{c}_{h}"
                                )
                                nc.scalar.activation(
                                    e_t[:, 0:w],
                                    sc_ps[:, 0:w],
                                    mybir.ActivationFunctionType.Exp,
                                    scale=1.0 / HD,
                                )
                                if r >= 0:
                                    nc.vector.tensor_mul(
                                        e_t[:, 0:128], e_t[:, 0:128], masksb
                                    )
                                e_ts[h] = e_t
                            for h in range(hq):
                                nc.tensor.matmul(
                                    at_tiles[h][:, off:ST],
                                    v_sb[:, c * 128 : (c + 1) * 128],
                                    e_ts[h][:, 0:w],
                                    start=(c == 0),
                                    stop=(c == nk - 1),
                                )
                            # 4 single-row denominator matmuls in distinct
                            # col-groups: HW runs them concurrently
                            for h in range(hq):
                                nc.tensor.matmul(
                                    den4[32 * h : 32 * h + 1, off:ST],
                                    onescsb,
                                    e_ts[h][:, 0:w],
                                    start=(c == 0),
                                    stop=(c == nk - 1),
                                    tile_position=(0, 32 * h),
                                )
                        # one strided reciprocal for all 4 heads' denominators
                        recip = sp.tile([128, ST], F32, tag="recip", name=f"recip{qt}")
                        nc.vector.reciprocal(recip, den4)
                        recipb = sp.tile([128, ST], BF16, tag="recipb", name=f"recipb{qt}")
                        nc.scalar.copy(recipb, recip)
                        for hh in range(hq):
                            bc_ps = psS.tile(
                                [128, ST], F32, tag="sc", name=f"bc{qt}_{hh}"
                            )
                            nc.tensor.matmul(
                                bc_ps,
                                onescbsb[32 * hh : 32 * hh + 1, :],
                                recipb[32 * hh : 32 * hh + 1, :],
                                start=True,
                                stop=True,
                                tile_position=(32 * hh, 0),
                            )
                            bc_sb = sp.tile(
                                [128, ST], F32, tag="bcsb", name=f"bcsb{qt}_{hh}"
                            )
                            nc.scalar.copy(bc_sb, bc_ps)
                            atn = atp.tile([128, ST], BF16, tag="attnT")
                            nc.vector.tensor_mul(atn, at_tiles[hh], bc_sb)
                            attn_tiles[hh] = atn
                    if prev_wo is not None:
                        emit_wo(*prev_wo)
                    prev_wo = (qt, attn_tiles)
                emit_wo(*prev_wo)
    return _legalize_single_wait(nc)


def host_prep(x, wq, wk, wv, wo, s=S, d=D, hq=HQ, ncores=NCORES):
    """Shared tensors + per-core weight shards, all host-side numpy."""
    scale = attn_scale(s, HD, MULT)
    xT = np.ascontiguousarray(x.reshape(s, d).T).astype(NPBF16)

    freq = ROPE_BASE ** (-(np.arange(0, HD, 2, dtype=np.float64) / HD))
    pos = np.arange(s, dtype=np.float64)
    angle = pos[:, None] * freq[None, :]  # [s, 64]
    cos = np.cos(angle).astype(NPBF16).T  # [64, s]
    sin = np.sin(angle).astype(NPBF16).T
    cosF = np.ascontiguousarray(np.concatenate([cos, cos], axis=0))
    sinSg = np.ascontiguousarray(np.concatenate([-sin, sin], axis=0))

    # triangular causal mask for diagonal chunks: keep iff p <= f
    p = np.arange(128)[:, None]
    f = np.arange(128)[None, :]
    maskT = (p <= f).astype(NPBF16)  # [128, 128]

    ident = np.eye(128, dtype=NPBF16)
    onesc = np.ones((128, 1), dtype=NPBF16)
    onesr = np.ones((128, 128), dtype=NPBF16)

    shared = dict(
        xT=xT, cosF=cosF, sinSg=sinSg, maskT=maskT, ident=ident, onesc=onesc,
        onesr=onesr,
    )

    in_maps = []
    for c in range(ncores):
        wq_c = wq[c * hq * 128 : (c + 1) * hq * 128, :]  # [hq*128, d]
        wk_c = wk[c * 128 : (c + 1) * 128, :]
        wv_c = wv[c * 128 : (c + 1) * 128, :] * scale
        wqkvT = np.ascontiguousarray(
            np.concatenate([wq_c.T, wk_c.T, wv_c.T], axis=1)
        ).astype(NPBF16)  # [d, (hq+2)*128]
        wo_c = wo[:, c * hq * 128 : (c + 1) * hq * 128]  # [d, hq*128]
        woT = np.ascontiguousarray(wo_c.T).astype(NPBF16)  # [hq*128, d]
        in_maps.append(dict(shared, wqkvT=wqkvT, woT=woT))
    return in_maps


_NC_CACHE = {}


def kernel(x, freqs_cis, wq, wk, wv, wo):
    del freqs_cis  # forward pass recomputes rope tables (matches reference)
    x = np.asarray(x, dtype=np.float32)
    key = (S, D, HQ)
    if key not in _NC_CACHE:
        _NC_CACHE[key] = build_core_kernel(S, D, HQ)
    nc = _NC_CACHE[key]
    in_maps = host_prep(
        x, np.asarray(wq, np.float32), np.asarray(wk, np.float32),
        np.asarray(wv, np.float32), np.asarray(wo, np.float32),
    )
    res = run_bass_kernel_spmd(nc, in_maps, core_ids=list(range(NCORES)))
    out = np.zeros((S, D), dtype=np.float32)
    for r in res.results:
        out += np.asarray(r["outp"], dtype=np.float32)
    return out.reshape(B, S, D)


if __name__ == "__main__":
    rng = np.random.default_rng(0)
    x = rng.standard_normal((B, S, D)).astype(np.float32)
    wq = (rng.standard_normal((H * HD, D)) * D**-0.5).astype(np.float32)
    wk = (rng.standard_normal((KVH * HD, D)) * D**-0.5).astype(np.float32)
    wv = (rng.standard_normal((KVH * HD, D)) * D**-0.5).astype(np.float32)
    wo = (rng.standard_normal((D, H * HD)) * (H * HD) ** -0.5).astype(np.float32)
    fc = rng.standard_normal((S, HD // 2)).astype(np.float32)
    out = kernel(x, fc, wq, wk, wv, wo)
    print(out.shape, out.dtype, np.abs(out).max())


# revision 22
# speedup vs baseline: 1.0529x; 1.0529x over previous
"""GQA attention layer (B=1, S=2048, D=4096, H=32, KVH=8, HD=128) on 8 TRN2
NeuronCores, tensor-parallel over heads.

Each core computes 4 query heads + their shared kv head end-to-end:
QKV projection -> RoPE -> causal attention (no-max-sub softmax, scores are
tiny) -> its slice of the wo projection. The 8 partial [S, D] outputs are
summed on the host (the "all-reduce after wo" of the sharding hint).

Device layouts (everything bf16 into the PE, fp32 PSUM accumulation):
  QT/KT  [HD=128(part), S]    from  lhsT=w[d,:], rhs=xT[d, s-tile]
  V      [S(part), HD]        via PE-transpose of VT
  scoresT[k(part), q]         lhsT=KT chunk, rhs=QT tile
  E = exp(scoresT/128) bf16; causal diagonal via 0/1 mask multiply
  attnT  [HD(part), q]        lhsT=V chunk, rhs=E  (accumulated over k)
  denom  [1, q]               lhsT=ones[128,1], rhs=E (accumulated over k)
  attnT_norm = attnT * bcast(1/denom)   (PE outer-product broadcast)
  out    [s(part), n]         lhsT=attnT_norm chunk, rhs=woT
"""

import json
import math

import ml_dtypes
import numpy as np

import concourse.bass as bass
import concourse.tile as tile
from concourse import mybir
from concourse.bass_utils import run_bass_kernel_spmd

BF16 = mybir.dt.bfloat16
F32 = mybir.dt.float32
FP8 = mybir.dt.float8e4
NPBF16 = ml_dtypes.bfloat16
NPFP8 = ml_dtypes.float8_e4m3

# Full problem constants
B, S, D = 1, 2048, 4096
H, KVH = 32, 8
HD = 128
NCORES = 8
HQ = H // NCORES  # query heads per core
MULT = 1.0
ROPE_BASE = 10000.0
ST = 512  # s-tile (PSUM bank width in fp32)


def attn_scale(seq_len=S, d_head=HD, mult=MULT):
    alpha = 1.0 / (1.0 + 4.0 * d_head / mult**2)
    lower = (math.log(seq_len) / seq_len) ** 0.5
    interp = math.exp((1.0 - alpha) * math.log(lower))
    return 1.0 / interp


def _legalize_single_wait(nc):
    """The walrus build in this container accepts only ONE sync wait per
    instruction ("Too many sync wait commands" in setupSyncWait). Split
    extra waits into preceding single-wait Drains (lowered to CTRL NOPs)
    on the same engine — same in-order stall semantics."""
    bir = json.loads(nc.to_json_bytes())
    ctr = 0
    for fn in bir["functions"]:
        for blk in fn["blocks"]:
            out = []
            for inst in blk["instructions"]:
                si = inst.get("sync_info")
                waits = (si or {}).get("on_wait") or []
                if len(waits) > 1:
                    for w in waits[:-1]:
                        ctr += 1
                        out.append(
                            {
                                "debug": inst.get("debug", 0),
                                "engine": inst["engine"],
                                "ins": [],
                                "name": f"{inst['name']}-mw{ctr}",
                                "opcode": "Drain",
                                "outs": [],
                                "sync_info": {"on_update": [], "on_wait": [w]},
                            }
                        )
                    si["on_wait"] = [waits[-1]]
                out.append(inst)
            blk["instructions"] = out
    fixed = json.dumps(bir).encode()
    nc.to_json_bytes = lambda: fixed
    return nc


def build_core_kernel(s=S, d=D, hq=HQ):
    """Bass module for one core: hq query heads + 1 kv head."""
    nst = s // ST  # s-tiles of 512
    ndk = d // 128  # contraction chunks
    nh = hq + 2  # q heads + k + v
    nnt = d // ST  # output n-tiles

    nqk = hq + 1  # q heads + k (fp8 path)

    nc = bass.Bass()
    xT_d = nc.dram_tensor("xT", [d, s], BF16, kind="ExternalInput")
    xT8_d = nc.dram_tensor("xT8", [d, s], FP8, kind="ExternalInput")
    wqk8_d = nc.dram_tensor("wqk8", [d, nqk * 128], FP8, kind="ExternalInput")
    wvT_d = nc.dram_tensor("wvT", [d, 128], BF16, kind="ExternalInput")
    woT_d = nc.dram_tensor("woT", [hq * 128, d], BF16, kind="ExternalInput")
    cosF_d = nc.dram_tensor("cosF", [128, s], BF16, kind="ExternalInput")
    sinSg_d = nc.dram_tensor("sinSg", [128, s], BF16, kind="ExternalInput")
    maskT_d = nc.dram_tensor("maskT", [128, 128], BF16, kind="ExternalInput")
    ident_d = nc.dram_tensor("ident", [128, 128], BF16, kind="ExternalInput")
    onesc_d = nc.dram_tensor("onesc", [128, 1], BF16, kind="ExternalInput")
    onesr_d = nc.dram_tensor("onesr", [128, 128], BF16, kind="ExternalInput")
    outp_d = nc.dram_tensor("outp", [s, d], BF16, kind="ExternalOutput")

    with tile.TileContext(nc) as tc:
        with (
            tc.tile_pool(name="const", bufs=1) as cp,
            tc.tile_pool(name="qkvsb", bufs=1) as qp,
            tc.tile_pool(name="xp", bufs=3) as xp,
            tc.tile_pool(name="rp", bufs=2) as rp,
            tc.tile_pool(name="vp", bufs=2) as vp,
            tc.tile_pool(name="ep", bufs=12) as ep,
            tc.tile_pool(name="sp", bufs=2) as sp,
            tc.tile_pool(name="op", bufs=6) as op,
            tc.tile_pool(name="at", bufs=8) as atp,
        ):
            # ---- resident constants ----
            # per-chunk weight tiles so the first matmul starts after the
            # first small DMA, not after the whole 10MB weight load
            npair = ndk // 2  # 256-row contraction pair-chunks (DoubleRow)
            w8g = [
                cp.tile([128, 4, 2, nqk * 128], FP8, tag=f"w8{g}", name=f"w8{g}")
                for g in range(npair // 4)
            ]
            wqk8_r = wqk8_d.rearrange("(g j ko p) m -> g p j ko m", j=4, ko=2, p=128)
            for g in range(npair // 4):
                nc.gpsimd.dma_start(w8g[g], wqk8_r[g])
            w8 = [w8g[j // 4][:, j % 4, :, :] for j in range(npair)]
            nwg = ndk // 8  # V weight groups of 8 contraction chunks
            wvsb4 = [
                cp.tile([128, 8, 128], BF16, tag=f"wv{g}", name=f"wv{g}")
                for g in range(nwg)
            ]
            wv_r = wvT_d.rearrange("(g c p) n -> g p c n", c=8, p=128)
            for g in range(nwg):
                nc.gpsimd.dma_start(wvsb4[g], wv_r[g])
            wvsb = [wvsb4[dk // 8][:, dk % 8, :] for dk in range(ndk)]
            cossb = cp.tile([128, s], BF16)
            nc.gpsimd.dma_start(cossb, cosF_d[:])
            sinsb = cp.tile([128, s], BF16)
            nc.gpsimd.dma_start(sinsb, sinSg_d[:])
            masksb = cp.tile([128, 128], BF16)
            nc.gpsimd.dma_start(masksb, maskT_d[:])
            identsb = cp.tile([128, 128], BF16)
            nc.gpsimd.dma_start(identsb, ident_d[:])
            onescsb = cp.tile([128, 1], BF16)
            nc.gpsimd.dma_start(onescsb, onesc_d[:])
            onescbsb = cp.tile([128, 128], BF16)
            nc.gpsimd.dma_start(onescbsb, onesr_d[:])
            wosb = [
                cp.tile([128, d], BF16, tag=f"wo{mh}", name=f"wo{mh}")
                for mh in range(hq)
            ]
            for mh in range(hq):
                nc.gpsimd.dma_start(wosb[mh], woT_d[mh * 128 : (mh + 1) * 128, :])

            # ---- persistent activations (bf16) ----
            qt_sb = [
                qp.tile([128, s], BF16, tag=f"QT{h}", name=f"QT{h}")
                for h in range(hq)
            ]
            kt_sb = qp.tile([128, s], BF16, tag="KT")
            v_sb = qp.tile([128, s], BF16, tag="V")  # [s%128 part, (s//128)*HD]

            # ================= phase A: QKV projection + RoPE =================
            with (
                tc.tile_pool(name="psA", bufs=7, space="PSUM") as psA,
                tc.tile_pool(name="psT", bufs=1, space="PSUM") as psT,
            ):
                nq = 4  # quarters per s-tile
                ndkq = ndk // nq  # bf16 contraction chunks per quarter (V)
                npq = npair // nq  # fp8 pair-chunks per quarter (QK)
                for st in range(nst):
                    ssl = slice(st * ST, (st + 1) * ST)
                    acc = [
                        psA.tile([128, ST], F32, tag="acc", name=f"acc{h}")
                        for h in range(nh)
                    ]
                    # heads-major over resident xT quarters: at the next s-tile
                    # boundary only acc[0] must be free for PE to proceed
                    for quar in range(nq):
                        x8a = xp.tile([128, npq, 2, ST], FP8, tag="x8")
                        for i in range(npq):
                            j = quar * npq + i
                            nc.sync.dma_start(
                                x8a[:, i, :, :],
                                xT8_d[j * 256 : (j + 1) * 256, ssl].rearrange(
                                    "(ko p) n -> p ko n", ko=2
                                ),
                            )
                        xta = xp.tile([128, ndkq, ST], BF16, tag="xT")
                        for dk in range(ndkq):
                            nc.sync.dma_start(
                                xta[:, dk, :],
                                xT_d[
                                    (quar * ndkq + dk) * 128 : (quar * ndkq + dk + 1)
                                    * 128,
                                    ssl,
                                ],
                            )
                        # Q + K: fp8 DoubleRow, 256-deep contraction per matmul
                        for h in range(nqk):
                            for i in range(npq):
                                nc.tensor.matmul(
                                    acc[h],
                                    w8[quar * npq + i][:, :, h * 128 : (h + 1) * 128],
                                    x8a[:, i, :, :],
                                    start=(quar == 0 and i == 0),
                                    stop=(quar == nq - 1 and i == npq - 1),
                                    perf_mode=mybir.MatmulPerfMode.DoubleRow,
                                )
                        # V: bf16
                        for dk in range(ndkq):
                            nc.tensor.matmul(
                                acc[nh - 1],
                                wvsb[quar * ndkq + dk],
                                xta[:, dk, :],
                                start=(quar == 0 and dk == 0),
                                stop=(quar == nq - 1 and dk == ndkq - 1),
                            )
                    # RoPE for q heads and k; write bf16
                    for h in range(hq + 1):
                        dst = qt_sb[h] if h < hq else kt_sb
                        t1 = rp.tile([128, ST], F32, tag="t1")
                        nc.vector.tensor_mul(t1, acc[h], cossb[:, ssl])
                        tsw = rp.tile([128, ST], F32, tag="tsw")
                        nc.scalar.copy(tsw[0:64, :], acc[h][64:128, :])
                        nc.scalar.copy(tsw[64:128, :], acc[h][0:64, :])
                        nc.vector.tensor_mul(tsw, tsw, sinsb[:, ssl])
                        nc.vector.tensor_add(dst[:, ssl], t1, tsw)
                    # V: transpose [HD, s-tile] -> [s-chunk, HD] blocks
                    for j in range(ST // 128):
                        vtmp = vp.tile([128, 128], BF16, tag="vtmp")
                        nc.scalar.copy(vtmp, acc[hq + 1][:, j * 128 : (j + 1) * 128])
                        tp_ps = psT.tile([128, 128], BF16, tag="tp")
                        nc.tensor.transpose(tp_ps, vtmp, identsb)
                        sc = st * (ST // 128) + j
                        nc.vector.tensor_copy(
                            v_sb[:, sc * 128 : (sc + 1) * 128], tp_ps
                        )

            # ============ phase B: attention + output projection ============
            with (
                tc.tile_pool(name="psS", bufs=3, space="PSUM") as psS,
                tc.tile_pool(name="psD", bufs=1, space="PSUM") as psD,
                tc.tile_pool(name="psAt", bufs=4, space="PSUM") as psAt,
            ):

                def emit_wo(qt, attn_tiles):
                    # wo for the s-chunks of q-tile qt (emitted one q-tile
                    # late so the normalize tail overlaps the next q-tile's
                    # attention matmuls)
                    with nc.named_scope(f"wo{qt}"):
                        for j in range(ST // 128):
                            sc = qt * (ST // 128) + j
                            for nt in range(nnt):
                                o_ps = psS.tile(
                                    [128, ST], F32, tag="sc",
                                    name=f"wo{qt}_{j}_{nt}",
                                )
                                for mh in range(hq):
                                    nc.tensor.matmul(
                                        o_ps,
                                        attn_tiles[mh][:, j * 128 : (j + 1) * 128],
                                        wosb[mh][:, nt * ST : (nt + 1) * ST],
                                        start=(mh == 0),
                                        stop=(mh == hq - 1),
                                    )
                                osb = op.tile([128, ST], BF16, tag="osb")
                                if (j + nt) % 2 == 0:
                                    nc.vector.tensor_copy(osb, o_ps)
                                else:
                                    nc.scalar.copy(osb, o_ps)
                                nc.sync.dma_start(
                                    outp_d[
                                        sc * 128 : (sc + 1) * 128,
                                        nt * ST : (nt + 1) * ST,
                                    ],
                                    osb,
                                )

                prev_wo = None
                for qt in range(nst):
                    nk = (qt + 1) * (ST // 128)  # causal: k chunks this q-tile
                    attn_tiles = {}
                    with nc.named_scope(f"attn{qt}"):
                        # one denominator bank per q-tile: head h accumulates
                        # into partition row 32*h (distinct col-groups)
                        den4 = psD.tile([128, ST], F32, tag="den")
                        nc.vector.memset(den4, 1.0)
                        at_tiles = {
                            h: psAt.tile([128, ST], F32, tag="at", name=f"at{qt}_{h}")
                            for h in range(hq)
                        }
                        for c in range(nk):
                            # diagonal chunks: only columns >= 128*r valid
                            r = c - (nk - 4)
                            off = 128 * r if r > 0 else 0
                            w = ST - off
                            e_ts = {}
                            for h in range(hq):
                                sc_ps = psS.tile(
                                    [128, ST], F32, tag="sc", name=f"sc{qt}_{c}_{h}"
                                )
                                nc.tensor.matmul(
                                    sc_ps[:, 0:w],
                                    kt_sb[:, c * 128 : (c + 1) * 128],
                                    qt_sb[h][:, qt * ST + off : (qt + 1) * ST],
                                    start=True,
                                    stop=True,
                                )
                                e_t = ep.tile(
                                    [128, ST], BF16, tag="E", name=f"e{qt}_{c}_{h}"
                                )
                                nc.scalar.activation(
                                    e_t[:, 0:w],
                                    sc_ps[:, 0:w],
                                    mybir.ActivationFunctionType.Exp,
                                    scale=1.0 / HD,
                                )
                                if r >= 0:
                                    nc.vector.tensor_mul(
                                        e_t[:, 0:128], e_t[:, 0:128], masksb
                                    )
                                e_ts[h] = e_t
                            for h in range(hq):
                                nc.tensor.matmul(
                                    at_tiles[h][:, off:ST],
                                    v_sb[:, c * 128 : (c + 1) * 128],
                                    e_ts[h][:, 0:w],
                                    start=(c == 0),
                                    stop=(c == nk - 1),
                                )
                            # 4 single-row denominator matmuls in distinct
                            # col-groups: HW runs them concurrently
                            for h in range(hq):
                                nc.tensor.matmul(
                                    den4[32 * h : 32 * h + 1, off:ST],
                                    onescsb,
                                    e_ts[h][:, 0:w],
                                    start=(c == 0),
                                    stop=(c == nk - 1),
                                    tile_position=(0, 32 * h),
                                )
                        # one strided reciprocal for all 4 heads' denominators
                        recip = sp.tile([128, ST], F32, tag="recip", name=f"recip{qt}")
                        nc.vector.reciprocal(recip, den4)
                        recipb = sp.tile([128, ST], BF16, tag="recipb", name=f"recipb{qt}")
                        nc.scalar.copy(recipb, recip)
                        for hh in range(hq):
                            bc_ps = psS.tile(
                                [128, ST], F32, tag="sc", name=f"bc{qt}_{hh}"
                            )
                            nc.tensor.matmul(
                                bc_ps,
                                onescbsb[32 * hh : 32 * hh + 1, :],
                                recipb[32 * hh : 32 * hh + 1, :],
                                start=True,
                                stop=True,
                                tile_position=(32 * hh, 0),
                            )
                            bc_sb = sp.tile(
                                [128, ST], F32, tag="bcsb", name=f"bcsb{qt}_{hh}"
                            )
                            nc.scalar.copy(bc_sb, bc_ps)
                            atn = atp.tile([128, ST], BF16, tag="attnT")
                            nc.vector.tensor_mul(atn, at_tiles[hh], bc_sb)
                            attn_tiles[hh] = atn
                    if prev_wo is not None:
                        emit_wo(*prev_wo)
                    prev_wo = (qt, attn_tiles)
                emit_wo(*prev_wo)
    return _legalize_single_wait(nc)


def host_prep(x, wq, wk, wv, wo, s=S, d=D, hq=HQ, ncores=NCORES):
    """Shared tensors + per-core weight shards, all host-side numpy."""
    scale = attn_scale(s, HD, MULT)
    xTf = np.ascontiguousarray(x.reshape(s, d).T)
    xT = xTf.astype(NPBF16)
    xT8 = xTf.astype(NPFP8)

    freq = ROPE_BASE ** (-(np.arange(0, HD, 2, dtype=np.float64) / HD))
    pos = np.arange(s, dtype=np.float64)
    angle = pos[:, None] * freq[None, :]  # [s, 64]
    cos = np.cos(angle).astype(NPBF16).T  # [64, s]
    sin = np.sin(angle).astype(NPBF16).T
    cosF = np.ascontiguousarray(np.concatenate([cos, cos], axis=0))
    sinSg = np.ascontiguousarray(np.concatenate([-sin, sin], axis=0))

    # triangular causal mask for diagonal chunks: keep iff p <= f
    p = np.arange(128)[:, None]
    f = np.arange(128)[None, :]
    maskT = (p <= f).astype(NPBF16)  # [128, 128]

    ident = np.eye(128, dtype=NPBF16)
    onesc = np.ones((128, 1), dtype=NPBF16)
    onesr = np.ones((128, 128), dtype=NPBF16)

    shared = dict(
        xT=xT, xT8=xT8, cosF=cosF, sinSg=sinSg, maskT=maskT, ident=ident,
        onesc=onesc, onesr=onesr,
    )

    in_maps = []
    for c in range(ncores):
        wq_c = wq[c * hq * 128 : (c + 1) * hq * 128, :]  # [hq*128, d]
        wk_c = wk[c * 128 : (c + 1) * 128, :]
        wv_c = wv[c * 128 : (c + 1) * 128, :] * scale
        wqk8 = np.ascontiguousarray(
            np.concatenate([wq_c.T, wk_c.T], axis=1)
        ).astype(NPFP8)  # [d, (hq+1)*128]
        wvT = np.ascontiguousarray(wv_c.T).astype(NPBF16)  # [d, 128]
        wo_c = wo[:, c * hq * 128 : (c + 1) * hq * 128]  # [d, hq*128]
        woT = np.ascontiguousarray(wo_c.T).astype(NPBF16)  # [hq*128, d]
        in_maps.append(dict(shared, wqk8=wqk8, wvT=wvT, woT=woT))
    return in_maps


_NC_CACHE = {}


def kernel(x, freqs_cis, wq, wk, wv, wo):
    del freqs_cis  # forward pass recomputes rope tables (matches reference)
    x = np.asarray(x, dtype=np.float32)
    key = (S, D, HQ)
    if key not in _NC_CACHE:
        _NC_CACHE[key] = build_core_kernel(S, D, HQ)
    nc = _NC_CACHE[key]
    in_maps = host_prep(
        x, np.asarray(wq, np.float32), np.asarray(wk, np.float32),
        np.asarray(wv, np.float32), np.asarray(wo, np.float32),
    )
    res = run_bass_kernel_spmd(nc, in_maps, core_ids=list(range(NCORES)))
    out = np.zeros((S, D), dtype=np.float32)
    for r in res.results:
        out += np.asarray(r["outp"], dtype=np.float32)
    return out.reshape(B, S, D)


if __name__ == "__main__":
    rng = np.random.default_rng(0)
    x = rng.standard_normal((B, S, D)).astype(np.float32)
    wq = (rng.standard_normal((H * HD, D)) * D**-0.5).astype(np.float32)
    wk = (rng.standard_normal((KVH * HD, D)) * D**-0.5).astype(np.float32)
    wv = (rng.standard_normal((KVH * HD, D)) * D**-0.5).astype(np.float32)
    wo = (rng.standard_normal((D, H * HD)) * (H * HD) ** -0.5).astype(np.float32)
    fc = rng.standard_normal((S, HD // 2)).astype(np.float32)
    out = kernel(x, fc, wq, wk, wv, wo)
    print(out.shape, out.dtype, np.abs(out).max())


# revision 23
# speedup vs baseline: 1.1103x; 1.0545x over previous
"""GQA attention layer (B=1, S=2048, D=4096, H=32, KVH=8, HD=128) on 8 TRN2
NeuronCores, tensor-parallel over heads.

Each core computes 4 query heads + their shared kv head end-to-end:
QKV projection -> RoPE -> causal attention (no-max-sub softmax, scores are
tiny) -> its slice of the wo projection. The 8 partial [S, D] outputs are
summed on the host (the "all-reduce after wo" of the sharding hint).

Device layouts (everything bf16 into the PE, fp32 PSUM accumulation):
  QT/KT  [HD=128(part), S]    from  lhsT=w[d,:], rhs=xT[d, s-tile]
  V      [S(part), HD]        via PE-transpose of VT
  scoresT[k(part), q]         lhsT=KT chunk, rhs=QT tile
  E = exp(scoresT/128) bf16; causal diagonal via 0/1 mask multiply
  attnT  [HD(part), q]        lhsT=V chunk, rhs=E  (accumulated over k)
  denom  [1, q]               lhsT=ones[128,1], rhs=E (accumulated over k)
  attnT_norm = attnT * bcast(1/denom)   (PE outer-product broadcast)
  out    [s(part), n]         lhsT=attnT_norm chunk, rhs=woT
"""

import json
import math

import ml_dtypes
import numpy as np

import concourse.bass as bass
import concourse.tile as tile
from concourse import mybir
from concourse.bass_utils import run_bass_kernel_spmd

BF16 = mybir.dt.bfloat16
F32 = mybir.dt.float32
FP8 = mybir.dt.float8e4
NPBF16 = ml_dtypes.bfloat16
NPFP8 = ml_dtypes.float8_e4m3

# Full problem constants
B, S, D = 1, 2048, 4096
H, KVH = 32, 8
HD = 128
NCORES = 8
HQ = H // NCORES  # query heads per core
MULT = 1.0
ROPE_BASE = 10000.0
ST = 512  # s-tile (PSUM bank width in fp32)


def attn_scale(seq_len=S, d_head=HD, mult=MULT):
    alpha = 1.0 / (1.0 + 4.0 * d_head / mult**2)
    lower = (math.log(seq_len) / seq_len) ** 0.5
    interp = math.exp((1.0 - alpha) * math.log(lower))
    return 1.0 / interp


def _legalize_single_wait(nc):
    """The walrus build in this container accepts only ONE sync wait per
    instruction ("Too many sync wait commands" in setupSyncWait). Split
    extra waits into preceding single-wait Drains (lowered to CTRL NOPs)
    on the same engine — same in-order stall semantics."""
    bir = json.loads(nc.to_json_bytes())
    ctr = 0
    for fn in bir["functions"]:
        for blk in fn["blocks"]:
            out = []
            for inst in blk["instructions"]:
                si = inst.get("sync_info")
                waits = (si or {}).get("on_wait") or []
                if len(waits) > 1:
                    for w in waits[:-1]:
                        ctr += 1
                        out.append(
                            {
                                "debug": inst.get("debug", 0),
                                "engine": inst["engine"],
                                "ins": [],
                                "name": f"{inst['name']}-mw{ctr}",
                                "opcode": "Drain",
                                "outs": [],
                                "sync_info": {"on_update": [], "on_wait": [w]},
                            }
                        )
                    si["on_wait"] = [waits[-1]]
                out.append(inst)
            blk["instructions"] = out
    fixed = json.dumps(bir).encode()
    nc.to_json_bytes = lambda: fixed
    return nc


def build_core_kernel(s=S, d=D, hq=HQ):
    """Bass module for one core: hq query heads + 1 kv head."""
    nst = s // ST  # s-tiles of 512
    ndk = d // 128  # contraction chunks
    nh = hq + 2  # q heads + k + v
    nnt = d // ST  # output n-tiles

    nqk = hq + 1  # q heads + k (fp8 path)

    nc = bass.Bass()
    xT_d = nc.dram_tensor("xT", [d, s], BF16, kind="ExternalInput")
    xT8_d = nc.dram_tensor("xT8", [d, s], FP8, kind="ExternalInput")
    wqk8_d = nc.dram_tensor("wqk8", [d, nqk * 128], FP8, kind="ExternalInput")
    wvT_d = nc.dram_tensor("wvT", [d, 128], BF16, kind="ExternalInput")
    woT_d = nc.dram_tensor("woT", [hq * 128, d], BF16, kind="ExternalInput")
    cosF_d = nc.dram_tensor("cosF", [128, s], BF16, kind="ExternalInput")
    sinSg_d = nc.dram_tensor("sinSg", [128, s], BF16, kind="ExternalInput")
    maskT_d = nc.dram_tensor("maskT", [128, 128], BF16, kind="ExternalInput")
    ident_d = nc.dram_tensor("ident", [128, 128], BF16, kind="ExternalInput")
    onesc_d = nc.dram_tensor("onesc", [128, 1], BF16, kind="ExternalInput")
    onesr_d = nc.dram_tensor("onesr", [128, 128], BF16, kind="ExternalInput")
    outp_d = nc.dram_tensor("outp", [s, d], BF16, kind="ExternalOutput")

    with tile.TileContext(nc) as tc:
        with (
            tc.tile_pool(name="const", bufs=1) as cp,
            tc.tile_pool(name="qkvsb", bufs=1) as qp,
            tc.tile_pool(name="xp", bufs=3) as xp,
            tc.tile_pool(name="rp", bufs=2) as rp,
            tc.tile_pool(name="vp", bufs=2) as vp,
            tc.tile_pool(name="ep", bufs=12) as ep,
            tc.tile_pool(name="sp", bufs=2) as sp,
            tc.tile_pool(name="op", bufs=6) as op,
            tc.tile_pool(name="at", bufs=8) as atp,
        ):
            # ---- resident constants ----
            # per-chunk weight tiles so the first matmul starts after the
            # first small DMA, not after the whole 10MB weight load
            npair = ndk // 2  # 256-row contraction pair-chunks (DoubleRow)
            w8g = [
                cp.tile([128, 4, 2, nqk * 128], FP8, tag=f"w8{g}", name=f"w8{g}")
                for g in range(npair // 4)
            ]
            wqk8_r = wqk8_d.rearrange("(g j ko p) m -> g p j ko m", j=4, ko=2, p=128)
            for g in range(npair // 4):
                nc.gpsimd.dma_start(w8g[g], wqk8_r[g])
            w8 = [w8g[j // 4][:, j % 4, :, :] for j in range(npair)]
            nwg = ndk // 8  # V weight groups of 8 contraction chunks
            wvsb4 = [
                cp.tile([128, 8, 128], BF16, tag=f"wv{g}", name=f"wv{g}")
                for g in range(nwg)
            ]
            wv_r = wvT_d.rearrange("(g c p) n -> g p c n", c=8, p=128)
            for g in range(nwg):
                nc.gpsimd.dma_start(wvsb4[g], wv_r[g])
            wvsb = [wvsb4[dk // 8][:, dk % 8, :] for dk in range(ndk)]
            cossb = cp.tile([128, s], BF16)
            nc.gpsimd.dma_start(cossb, cosF_d[:])
            sinsb = cp.tile([128, s], BF16)
            nc.gpsimd.dma_start(sinsb, sinSg_d[:])
            masksb = cp.tile([128, 128], BF16)
            nc.gpsimd.dma_start(masksb, maskT_d[:])
            identsb = cp.tile([128, 128], BF16)
            nc.gpsimd.dma_start(identsb, ident_d[:])
            onescsb = cp.tile([128, 1], BF16)
            nc.gpsimd.dma_start(onescsb, onesc_d[:])
            onescbsb = cp.tile([128, 128], BF16)
            nc.gpsimd.dma_start(onescbsb, onesr_d[:])
            wosb = [
                cp.tile([128, d], BF16, tag=f"wo{mh}", name=f"wo{mh}")
                for mh in range(hq)
            ]
            for mh in range(hq):
                nc.gpsimd.dma_start(wosb[mh], woT_d[mh * 128 : (mh + 1) * 128, :])

            # ---- persistent activations (bf16) ----
            qt_sb = [
                qp.tile([128, s], BF16, tag=f"QT{h}", name=f"QT{h}")
                for h in range(hq)
            ]
            kt_sb = qp.tile([128, s], BF16, tag="KT")
            v_sb = qp.tile([128, s], BF16, tag="V")  # [s%128 part, (s//128)*HD]

            # ================= phase A: QKV projection + RoPE =================
            with (
                tc.tile_pool(name="psA", bufs=7, space="PSUM") as psA,
                tc.tile_pool(name="psT", bufs=1, space="PSUM") as psT,
            ):
                nq = 4  # quarters per s-tile
                ndkq = ndk // nq  # bf16 contraction chunks per quarter (V)
                npq = npair // nq  # fp8 pair-chunks per quarter (QK)
                for st in range(nst):
                    ssl = slice(st * ST, (st + 1) * ST)
                    acc = [
                        psA.tile([128, ST], F32, tag="acc", name=f"acc{h}")
                        for h in range(nh)
                    ]
                    # heads-major over resident xT quarters: at the next s-tile
                    # boundary only acc[0] must be free for PE to proceed
                    for quar in range(nq):
                        x8a = xp.tile([128, npq, 2, ST], FP8, tag="x8")
                        nc.sync.dma_start(
                            x8a,
                            xT8_d[quar * npq * 256 : (quar + 1) * npq * 256, ssl]
                            .rearrange("(i ko p) n -> p i ko n", ko=2, p=128),
                        )
                        xta = xp.tile([128, ndkq, ST], BF16, tag="xT")
                        nc.sync.dma_start(
                            xta,
                            xT_d[quar * ndkq * 128 : (quar + 1) * ndkq * 128, ssl]
                            .rearrange("(dk p) n -> p dk n", p=128),
                        )
                        # Q + K: fp8 DoubleRow, 256-deep contraction per matmul
                        for h in range(nqk):
                            for i in range(npq):
                                nc.tensor.matmul(
                                    acc[h],
                                    w8[quar * npq + i][:, :, h * 128 : (h + 1) * 128],
                                    x8a[:, i, :, :],
                                    start=(quar == 0 and i == 0),
                                    stop=(quar == nq - 1 and i == npq - 1),
                                    perf_mode=mybir.MatmulPerfMode.DoubleRow,
                                )
                        # V: bf16
                        for dk in range(ndkq):
                            nc.tensor.matmul(
                                acc[nh - 1],
                                wvsb[quar * ndkq + dk],
                                xta[:, dk, :],
                                start=(quar == 0 and dk == 0),
                                stop=(quar == nq - 1 and dk == ndkq - 1),
                            )
                    # RoPE for q heads and k; write bf16
                    for h in range(hq + 1):
                        dst = qt_sb[h] if h < hq else kt_sb
                        t1 = rp.tile([128, ST], F32, tag="t1")
                        nc.vector.tensor_mul(t1, acc[h], cossb[:, ssl])
                        tsw = rp.tile([128, ST], F32, tag="tsw")
                        nc.scalar.copy(tsw[0:64, :], acc[h][64:128, :])
                        nc.scalar.copy(tsw[64:128, :], acc[h][0:64, :])
                        nc.vector.tensor_mul(tsw, tsw, sinsb[:, ssl])
                        nc.vector.tensor_add(dst[:, ssl], t1, tsw)
                    # V: transpose [HD, s-tile] -> [s-chunk, HD] blocks
                    for j in range(ST // 128):
                        vtmp = vp.tile([128, 128], BF16, tag="vtmp")
                        nc.scalar.copy(vtmp, acc[hq + 1][:, j * 128 : (j + 1) * 128])
                        tp_ps = psT.tile([128, 128], BF16, tag="tp")
                        nc.tensor.transpose(tp_ps, vtmp, identsb)
                        sc = st * (ST // 128) + j
                        nc.vector.tensor_copy(
                            v_sb[:, sc * 128 : (sc + 1) * 128], tp_ps
                        )

            # ============ phase B: attention + output projection ============
            with (
                tc.tile_pool(name="psS", bufs=3, space="PSUM") as psS,
                tc.tile_pool(name="psD", bufs=1, space="PSUM") as psD,
                tc.tile_pool(name="psAt", bufs=4, space="PSUM") as psAt,
            ):

                def emit_wo(qt, attn_tiles):
                    # wo for the s-chunks of q-tile qt (emitted one q-tile
                    # late so the normalize tail overlaps the next q-tile's
                    # attention matmuls)
                    with nc.named_scope(f"wo{qt}"):
                        for j in range(ST // 128):
                            sc = qt * (ST // 128) + j
                            for nt in range(nnt):
                                o_ps = psS.tile(
                                    [128, ST], F32, tag="sc",
                                    name=f"wo{qt}_{j}_{nt}",
                                )
                                for mh in range(hq):
                                    nc.tensor.matmul(
                                        o_ps,
                                        attn_tiles[mh][:, j * 128 : (j + 1) * 128],
                                        wosb[mh][:, nt * ST : (nt + 1) * ST],
                                        start=(mh == 0),
                                        stop=(mh == hq - 1),
                                    )
                                osb = op.tile([128, ST], BF16, tag="osb")
                                if (j + nt) % 2 == 0:
                                    nc.vector.tensor_copy(osb, o_ps)
                                else:
                                    nc.scalar.copy(osb, o_ps)
                                nc.sync.dma_start(
                                    outp_d[
                                        sc * 128 : (sc + 1) * 128,
                                        nt * ST : (nt + 1) * ST,
                                    ],
                                    osb,
                                )

                prev_wo = None
                for qt in range(nst):
                    nk = (qt + 1) * (ST // 128)  # causal: k chunks this q-tile
                    attn_tiles = {}
                    with nc.named_scope(f"attn{qt}"):
                        # one denominator bank per q-tile: head h accumulates
                        # into partition row 32*h (distinct col-groups)
                        den4 = psD.tile([128, ST], F32, tag="den")
                        nc.vector.memset(den4, 1.0)
                        at_tiles = {
                            h: psAt.tile([128, ST], F32, tag="at", name=f"at{qt}_{h}")
                            for h in range(hq)
                        }
                        for c in range(nk):
                            # diagonal chunks: only columns >= 128*r valid
                            r = c - (nk - 4)
                            off = 128 * r if r > 0 else 0
                            w = ST - off
                            e_ts = {}
                            for h in range(hq):
                                sc_ps = psS.tile(
                                    [128, ST], F32, tag="sc", name=f"sc{qt}_{c}_{h}"
                                )
                                nc.tensor.matmul(
                                    sc_ps[:, 0:w],
                                    kt_sb[:, c * 128 : (c + 1) * 128],
                                    qt_sb[h][:, qt * ST + off : (qt + 1) * ST],
                                    start=True,
                                    stop=True,
                                )
                                e_t = ep.tile(
                                    [128, ST], BF16, tag="E", name=f"e{qt}_{c}_{h}"
                                )
                                nc.scalar.activation(
                                    e_t[:, 0:w],
                                    sc_ps[:, 0:w],
                                    mybir.ActivationFunctionType.Exp,
                                    scale=1.0 / HD,
                                )
                                if r >= 0:
                                    nc.vector.tensor_mul(
                                        e_t[:, 0:128], e_t[:, 0:128], masksb
                                    )
                                e_ts[h] = e_t
                            for h in range(hq):
                                nc.tensor.matmul(
                                    at_tiles[h][:, off:ST],
                                    v_sb[:, c * 128 : (c + 1) * 128],
                                    e_ts[h][:, 0:w],
                                    start=(c == 0),
                                    stop=(c == nk - 1),
                                )
                            # 4 single-row denominator matmuls in distinct
                            # col-groups: HW runs them concurrently
                            for h in range(hq):
                                nc.tensor.matmul(
                                    den4[32 * h : 32 * h + 1, off:ST],
                                    onescsb,
                                    e_ts[h][:, 0:w],
                                    start=(c == 0),
                                    stop=(c == nk - 1),
                                    tile_position=(0, 32 * h),
                                )
                        # one strided reciprocal for all 4 heads' denominators
                        recip = sp.tile([128, ST], F32, tag="recip", name=f"recip{qt}")
                        nc.vector.reciprocal(recip, den4)
                        recipb = sp.tile([128, ST], BF16, tag="recipb", name=f"recipb{qt}")
                        nc.scalar.copy(recipb, recip)
                        for hh in range(hq):
                            bc_ps = psS.tile(
                                [128, ST], F32, tag="sc", name=f"bc{qt}_{hh}"
                            )
                            nc.tensor.matmul(
                                bc_ps,
                                onescbsb[32 * hh : 32 * hh + 1, :],
                                recipb[32 * hh : 32 * hh + 1, :],
                                start=True,
                                stop=True,
                                tile_position=(32 * hh, 0),
                            )
                            bc_sb = sp.tile(
                                [128, ST], F32, tag="bcsb", name=f"bcsb{qt}_{hh}"
                            )
                            nc.scalar.copy(bc_sb, bc_ps)
                            atn = atp.tile([128, ST], BF16, tag="attnT")
                            nc.vector.tensor_mul(atn, at_tiles[hh], bc_sb)
                            attn_tiles[hh] = atn
                    if prev_wo is not None:
                        emit_wo(*prev_wo)
                    prev_wo = (qt, attn_tiles)
                emit_wo(*prev_wo)
    return _legalize_single_wait(nc)


def host_prep(x, wq, wk, wv, wo, s=S, d=D, hq=HQ, ncores=NCORES):
    """Shared tensors + per-core weight shards, all host-side numpy."""
    scale = attn_scale(s, HD, MULT)
    xTf = np.ascontiguousarray(x.reshape(s, d).T)
    xT = xTf.astype(NPBF16)
    xT8 = xTf.astype(NPFP8)

    freq = ROPE_BASE ** (-(np.arange(0, HD, 2, dtype=np.float64) / HD))
    pos = np.arange(s, dtype=np.float64)
    angle = pos[:, None] * freq[None, :]  # [s, 64]
    cos = np.cos(angle).astype(NPBF16).T  # [64, s]
    sin = np.sin(angle).astype(NPBF16).T
    cosF = np.ascontiguousarray(np.concatenate([cos, cos], axis=0))
    sinSg = np.ascontiguousarray(np.concatenate([-sin, sin], axis=0))

    # triangular causal mask for diagonal chunks: keep iff p <= f
    p = np.arange(128)[:, None]
    f = np.arange(128)[None, :]
    maskT = (p <= f).astype(NPBF16)  # [128, 128]

    ident = np.eye(128, dtype=NPBF16)
    onesc = np.ones((128, 1), dtype=NPBF16)
    onesr = np.ones((128, 128), dtype=NPBF16)

    shared = dict(
        xT=xT, xT8=xT8, cosF=cosF, sinSg=sinSg, maskT=maskT, ident=ident,
        onesc=onesc, onesr=onesr,
    )

    in_maps = []
    for c in range(ncores):
        wq_c = wq[c * hq * 128 : (c + 1) * hq * 128, :]  # [hq*128, d]
        wk_c = wk[c * 128 : (c + 1) * 128, :]
        wv_c = wv[c * 128 : (c + 1) * 128, :] * scale
        wqk8 = np.ascontiguousarray(
            np.concatenate([wq_c.T, wk_c.T], axis=1)
        ).astype(NPFP8)  # [d, (hq+1)*128]
        wvT = np.ascontiguousarray(wv_c.T).astype(NPBF16)  # [d, 128]
        wo_c = wo[:, c * hq * 128 : (c + 1) * hq * 128]  # [d, hq*128]
        woT = np.ascontiguousarray(wo_c.T).astype(NPBF16)  # [hq*128, d]
        in_maps.append(dict(shared, wqk8=wqk8, wvT=wvT, woT=woT))
    return in_maps


_NC_CACHE = {}


def kernel(x, freqs_cis, wq, wk, wv, wo):
    del freqs_cis  # forward pass recomputes rope tables (matches reference)
    x = np.asarray(x, dtype=np.float32)
    key = (S, D, HQ)
    if key not in _NC_CACHE:
        _NC_CACHE[key] = build_core_kernel(S, D, HQ)
    nc = _NC_CACHE[key]
    in_maps = host_prep(
        x, np.asarray(wq, np.float32), np.asarray(wk, np.float32),
        np.asarray(wv, np.float32), np.asarray(wo, np.float32),
    )
    res = run_bass_kernel_spmd(nc, in_maps, core_ids=list(range(NCORES)))
    out = np.zeros((S, D), dtype=np.float32)
    for r in res.results:
        out += np.asarray(r["outp"], dtype=np.float32)
    return out.reshape(B, S, D)


if __name__ == "__main__":
    rng = np.random.default_rng(0)
    x = rng.standard_normal((B, S, D)).astype(np.float32)
    wq = (rng.standard_normal((H * HD, D)) * D**-0.5).astype(np.float32)
    wk = (rng.standard_normal((KVH * HD, D)) * D**-0.5).astype(np.float32)
    wv = (rng.standard_normal((KVH * HD, D)) * D**-0.5).astype(np.float32)
    wo = (rng.standard_normal((D, H * HD)) * (H * HD) ** -0.5).astype(np.float32)
    fc = rng.standard_normal((S, HD // 2)).astype(np.float32)
    out = kernel(x, fc, wq, wk, wv, wo)
    print(out.shape, out.dtype, np.abs(out).max())


# revision 24
# speedup vs baseline: 1.1426x; 1.0291x over previous
"""GQA attention layer (B=1, S=2048, D=4096, H=32, KVH=8, HD=128) on 8 TRN2
NeuronCores, tensor-parallel over heads.

Each core computes 4 query heads + their shared kv head end-to-end:
QKV projection -> RoPE -> causal attention (no-max-sub softmax, scores are
tiny) -> its slice of the wo projection. The 8 partial [S, D] outputs are
summed on the host (the "all-reduce after wo" of the sharding hint).

Device layouts (everything bf16 into the PE, fp32 PSUM accumulation):
  QT/KT  [HD=128(part), S]    from  lhsT=w[d,:], rhs=xT[d, s-tile]
  V      [S(part), HD]        via PE-transpose of VT
  scoresT[k(part), q]         lhsT=KT chunk, rhs=QT tile
  E = exp(scoresT/128) bf16; causal diagonal via 0/1 mask multiply
  attnT  [HD(part), q]        lhsT=V chunk, rhs=E  (accumulated over k)
  denom  [1, q]               lhsT=ones[128,1], rhs=E (accumulated over k)
  attnT_norm = attnT * bcast(1/denom)   (PE outer-product broadcast)
  out    [s(part), n]         lhsT=attnT_norm chunk, rhs=woT
"""

import json
import math

import ml_dtypes
import numpy as np

import concourse.bass as bass
import concourse.tile as tile
from concourse import mybir
from concourse.bass_utils import run_bass_kernel_spmd

BF16 = mybir.dt.bfloat16
F32 = mybir.dt.float32
FP8 = mybir.dt.float8e4
NPBF16 = ml_dtypes.bfloat16
NPFP8 = ml_dtypes.float8_e4m3

# Full problem constants
B, S, D = 1, 2048, 4096
H, KVH = 32, 8
HD = 128
NCORES = 8
HQ = H // NCORES  # query heads per core
MULT = 1.0
ROPE_BASE = 10000.0
ST = 512  # s-tile (PSUM bank width in fp32)


def attn_scale(seq_len=S, d_head=HD, mult=MULT):
    alpha = 1.0 / (1.0 + 4.0 * d_head / mult**2)
    lower = (math.log(seq_len) / seq_len) ** 0.5
    interp = math.exp((1.0 - alpha) * math.log(lower))
    return 1.0 / interp


def _legalize_single_wait(nc):
    """The walrus build in this container accepts only ONE sync wait per
    instruction ("Too many sync wait commands" in setupSyncWait). Split
    extra waits into preceding single-wait Drains (lowered to CTRL NOPs)
    on the same engine — same in-order stall semantics."""
    bir = json.loads(nc.to_json_bytes())
    ctr = 0
    for fn in bir["functions"]:
        for blk in fn["blocks"]:
            out = []
            for inst in blk["instructions"]:
                si = inst.get("sync_info")
                waits = (si or {}).get("on_wait") or []
                if len(waits) > 1:
                    for w in waits[:-1]:
                        ctr += 1
                        out.append(
                            {
                                "debug": inst.get("debug", 0),
                                "engine": inst["engine"],
                                "ins": [],
                                "name": f"{inst['name']}-mw{ctr}",
                                "opcode": "Drain",
                                "outs": [],
                                "sync_info": {"on_update": [], "on_wait": [w]},
                            }
                        )
                    si["on_wait"] = [waits[-1]]
                out.append(inst)
            blk["instructions"] = out
    fixed = json.dumps(bir).encode()
    nc.to_json_bytes = lambda: fixed
    return nc


def build_core_kernel(s=S, d=D, hq=HQ):
    """Bass module for one core: hq query heads + 1 kv head."""
    nst = s // ST  # s-tiles of 512
    ndk = d // 128  # contraction chunks
    nh = hq + 2  # q heads + k + v
    nnt = d // ST  # output n-tiles

    nqk = hq + 1  # q heads + k (fp8 path)

    nc = bass.Bass()
    xT_d = nc.dram_tensor("xT", [d, s], BF16, kind="ExternalInput")
    xT8_d = nc.dram_tensor("xT8", [d, s], FP8, kind="ExternalInput")
    wqk8_d = nc.dram_tensor("wqk8", [d, nqk * 128], FP8, kind="ExternalInput")
    wvT_d = nc.dram_tensor("wvT", [d, 128], BF16, kind="ExternalInput")
    woT_d = nc.dram_tensor("woT", [hq * 128, d], BF16, kind="ExternalInput")
    cosF_d = nc.dram_tensor("cosF", [128, s], BF16, kind="ExternalInput")
    sinSg_d = nc.dram_tensor("sinSg", [128, s], BF16, kind="ExternalInput")
    maskT_d = nc.dram_tensor("maskT", [128, 128], BF16, kind="ExternalInput")
    ident_d = nc.dram_tensor("ident", [128, 128], BF16, kind="ExternalInput")
    onesc_d = nc.dram_tensor("onesc", [128, 1], BF16, kind="ExternalInput")
    onesr_d = nc.dram_tensor("onesr", [128, 128], BF16, kind="ExternalInput")
    outp_d = nc.dram_tensor("outp", [s, d], BF16, kind="ExternalOutput")

    with tile.TileContext(nc) as tc:
        with (
            tc.tile_pool(name="const", bufs=1) as cp,
            tc.tile_pool(name="qkvsb", bufs=1) as qp,
            tc.tile_pool(name="xp", bufs=3) as xp,
            tc.tile_pool(name="rp", bufs=2) as rp,
            tc.tile_pool(name="vp", bufs=2) as vp,
            tc.tile_pool(name="ep", bufs=12) as ep,
            tc.tile_pool(name="sp", bufs=3) as sp,
            tc.tile_pool(name="op", bufs=8) as op,
            tc.tile_pool(name="at", bufs=8) as atp,
        ):
            # ---- resident constants ----
            # per-chunk weight tiles so the first matmul starts after the
            # first small DMA, not after the whole 10MB weight load
            npair = ndk // 2  # 256-row contraction pair-chunks (DoubleRow)
            w8g = [
                cp.tile([128, 4, 2, nqk * 128], FP8, tag=f"w8{g}", name=f"w8{g}")
                for g in range(npair // 4)
            ]
            wqk8_r = wqk8_d.rearrange("(g j ko p) m -> g p j ko m", j=4, ko=2, p=128)
            for g in range(npair // 4):
                nc.gpsimd.dma_start(w8g[g], wqk8_r[g])
            w8 = [w8g[j // 4][:, j % 4, :, :] for j in range(npair)]
            nwg = ndk // 8  # V weight groups of 8 contraction chunks
            wvsb4 = [
                cp.tile([128, 8, 128], BF16, tag=f"wv{g}", name=f"wv{g}")
                for g in range(nwg)
            ]
            wv_r = wvT_d.rearrange("(g c p) n -> g p c n", c=8, p=128)
            for g in range(nwg):
                nc.gpsimd.dma_start(wvsb4[g], wv_r[g])
            wvsb = [wvsb4[dk // 8][:, dk % 8, :] for dk in range(ndk)]
            cossb = cp.tile([128, s], BF16)
            nc.gpsimd.dma_start(cossb, cosF_d[:])
            sinsb = cp.tile([128, s], BF16)
            nc.gpsimd.dma_start(sinsb, sinSg_d[:])
            masksb = cp.tile([128, 128], BF16)
            nc.gpsimd.dma_start(masksb, maskT_d[:])
            identsb = cp.tile([128, 128], BF16)
            nc.gpsimd.dma_start(identsb, ident_d[:])
            onescsb = cp.tile([128, 1], BF16)
            nc.gpsimd.dma_start(onescsb, onesc_d[:])
            onescbsb = cp.tile([128, 128], BF16)
            nc.gpsimd.dma_start(onescbsb, onesr_d[:])
            wosb = [
                cp.tile([128, d], BF16, tag=f"wo{mh}", name=f"wo{mh}")
                for mh in range(hq)
            ]
            for mh in range(hq):
                nc.gpsimd.dma_start(wosb[mh], woT_d[mh * 128 : (mh + 1) * 128, :])

            # ---- persistent activations (bf16) ----
            qt_sb = [
                qp.tile([128, s], BF16, tag=f"QT{h}", name=f"QT{h}")
                for h in range(hq)
            ]
            kt_sb = qp.tile([128, s], BF16, tag="KT")
            v_sb = qp.tile([128, s], BF16, tag="V")  # [s%128 part, (s//128)*HD]

            # ================= phase A: QKV projection + RoPE =================
            with (
                tc.tile_pool(name="psA", bufs=7, space="PSUM") as psA,
                tc.tile_pool(name="psT", bufs=1, space="PSUM") as psT,
            ):
                nq = 4  # quarters per s-tile
                ndkq = ndk // nq  # bf16 contraction chunks per quarter (V)
                npq = npair // nq  # fp8 pair-chunks per quarter (QK)
                for st in range(nst):
                    ssl = slice(st * ST, (st + 1) * ST)
                    acc = [
                        psA.tile([128, ST], F32, tag="acc", name=f"acc{h}")
                        for h in range(nh)
                    ]
                    # heads-major over resident xT quarters: at the next s-tile
                    # boundary only acc[0] must be free for PE to proceed
                    for quar in range(nq):
                        x8a = xp.tile([128, npq, 2, ST], FP8, tag="x8")
                        nc.sync.dma_start(
                            x8a,
                            xT8_d[quar * npq * 256 : (quar + 1) * npq * 256, ssl]
                            .rearrange("(i ko p) n -> p i ko n", ko=2, p=128),
                        )
                        xta = xp.tile([128, ndkq, ST], BF16, tag="xT")
                        nc.sync.dma_start(
                            xta,
                            xT_d[quar * ndkq * 128 : (quar + 1) * ndkq * 128, ssl]
                            .rearrange("(dk p) n -> p dk n", p=128),
                        )
                        # Q + K: fp8 DoubleRow, 256-deep contraction per matmul
                        for h in range(nqk):
                            for i in range(npq):
                                nc.tensor.matmul(
                                    acc[h],
                                    w8[quar * npq + i][:, :, h * 128 : (h + 1) * 128],
                                    x8a[:, i, :, :],
                                    start=(quar == 0 and i == 0),
                                    stop=(quar == nq - 1 and i == npq - 1),
                                    perf_mode=mybir.MatmulPerfMode.DoubleRow,
                                )
                        # V: bf16
                        for dk in range(ndkq):
                            nc.tensor.matmul(
                                acc[nh - 1],
                                wvsb[quar * ndkq + dk],
                                xta[:, dk, :],
                                start=(quar == 0 and dk == 0),
                                stop=(quar == nq - 1 and dk == ndkq - 1),
                            )
                    # RoPE for q heads and k; write bf16
                    for h in range(hq + 1):
                        dst = qt_sb[h] if h < hq else kt_sb
                        t1 = rp.tile([128, ST], F32, tag="t1")
                        nc.vector.tensor_mul(t1, acc[h], cossb[:, ssl])
                        tsw = rp.tile([128, ST], F32, tag="tsw")
                        nc.vector.tensor_copy(tsw[0:64, :], acc[h][64:128, :])
                        nc.vector.tensor_copy(tsw[64:128, :], acc[h][0:64, :])
                        nc.vector.tensor_mul(tsw, tsw, sinsb[:, ssl])
                        nc.vector.tensor_add(dst[:, ssl], t1, tsw)
                    # V: transpose [HD, s-tile] -> [s-chunk, HD] blocks
                    for j in range(ST // 128):
                        vtmp = vp.tile([128, 128], BF16, tag="vtmp")
                        nc.scalar.copy(vtmp, acc[hq + 1][:, j * 128 : (j + 1) * 128])
                        tp_ps = psT.tile([128, 128], BF16, tag="tp")
                        nc.tensor.transpose(tp_ps, vtmp, identsb)
                        sc = st * (ST // 128) + j
                        nc.vector.tensor_copy(
                            v_sb[:, sc * 128 : (sc + 1) * 128], tp_ps
                        )

            # ============ phase B: attention + output projection ============
            with (
                tc.tile_pool(name="psS", bufs=3, space="PSUM") as psS,
                tc.tile_pool(name="psD", bufs=1, space="PSUM") as psD,
                tc.tile_pool(name="psAt", bufs=4, space="PSUM") as psAt,
            ):

                def emit_wo(qt, attn_tiles):
                    # wo for the s-chunks of q-tile qt (emitted one q-tile
                    # late so the normalize tail overlaps the next q-tile's
                    # attention matmuls)
                    with nc.named_scope(f"wo{qt}"):
                        for j in range(ST // 128):
                            sc = qt * (ST // 128) + j
                            for nt in range(nnt):
                                o_ps = psS.tile(
                                    [128, ST], F32, tag="sc",
                                    name=f"wo{qt}_{j}_{nt}",
                                )
                                for mh in range(hq):
                                    nc.tensor.matmul(
                                        o_ps,
                                        attn_tiles[mh][:, j * 128 : (j + 1) * 128],
                                        wosb[mh][:, nt * ST : (nt + 1) * ST],
                                        start=(mh == 0),
                                        stop=(mh == hq - 1),
                                    )
                                osb = op.tile([128, ST], BF16, tag="osb")
                                nc.vector.tensor_copy(osb, o_ps)
                                nc.sync.dma_start(
                                    outp_d[
                                        sc * 128 : (sc + 1) * 128,
                                        nt * ST : (nt + 1) * ST,
                                    ],
                                    osb,
                                )

                prev_wo = None
                for qt in range(nst):
                    nk = (qt + 1) * (ST // 128)  # causal: k chunks this q-tile
                    attn_tiles = {}
                    with nc.named_scope(f"attn{qt}"):
                        # one denominator bank per q-tile: head h accumulates
                        # into partition row 32*h (distinct col-groups)
                        den4 = psD.tile([128, ST], F32, tag="den")
                        nc.vector.memset(den4, 1.0)
                        at_tiles = {
                            h: psAt.tile([128, ST], F32, tag="at", name=f"at{qt}_{h}")
                            for h in range(hq)
                        }
                        for c in range(nk):
                            # diagonal chunks: only columns >= 128*r valid
                            r = c - (nk - 4)
                            off = 128 * r if r > 0 else 0
                            w = ST - off
                            e_ts = {}
                            for h in range(hq):
                                sc_ps = psS.tile(
                                    [128, ST], F32, tag="sc", name=f"sc{qt}_{c}_{h}"
                                )
                                nc.tensor.matmul(
                                    sc_ps[:, 0:w],
                                    kt_sb[:, c * 128 : (c + 1) * 128],
                                    qt_sb[h][:, qt * ST + off : (qt + 1) * ST],
                                    start=True,
                                    stop=True,
                                )
                                e_t = ep.tile(
                                    [128, ST], BF16, tag="E", name=f"e{qt}_{c}_{h}"
                                )
                                nc.scalar.activation(
                                    e_t[:, 0:w],
                                    sc_ps[:, 0:w],
                                    mybir.ActivationFunctionType.Exp,
                                    scale=1.0 / HD,
                                )
                                if r >= 0:
                                    nc.vector.tensor_mul(
                                        e_t[:, 0:128], e_t[:, 0:128], masksb
                                    )
                                e_ts[h] = e_t
                            for h in range(hq):
                                nc.tensor.matmul(
                                    at_tiles[h][:, off:ST],
                                    v_sb[:, c * 128 : (c + 1) * 128],
                                    e_ts[h][:, 0:w],
                                    start=(c == 0),
                                    stop=(c == nk - 1),
                                )
                            # 4 single-row denominator matmuls in distinct
                            # col-groups: HW runs them concurrently
                            for h in range(hq):
                                nc.tensor.matmul(
                                    den4[32 * h : 32 * h + 1, off:ST],
                                    onescsb,
                                    e_ts[h][:, 0:w],
                                    start=(c == 0),
                                    stop=(c == nk - 1),
                                    tile_position=(0, 32 * h),
                                )
                        # one strided reciprocal for all 4 heads' denominators
                        recip = sp.tile([128, ST], F32, tag="recip", name=f"recip{qt}")
                        nc.vector.reciprocal(recip, den4)
                        recipb = sp.tile([128, ST], BF16, tag="recipb", name=f"recipb{qt}")
                        nc.scalar.copy(recipb, recip)
                        for hh in range(hq):
                            bc_ps = psS.tile(
                                [128, ST], F32, tag="sc", name=f"bc{qt}_{hh}"
                            )
                            nc.tensor.matmul(
                                bc_ps,
                                onescbsb[32 * hh : 32 * hh + 1, :],
                                recipb[32 * hh : 32 * hh + 1, :],
                                start=True,
                                stop=True,
                                tile_position=(32 * hh, 0),
                            )
                            bc_sb = sp.tile(
                                [128, ST], F32, tag="bcsb", name=f"bcsb{qt}_{hh}"
                            )
                            nc.scalar.copy(bc_sb, bc_ps)
                            atn = atp.tile([128, ST], BF16, tag="attnT")
                            nc.vector.tensor_mul(atn, at_tiles[hh], bc_sb)
                            attn_tiles[hh] = atn
                    if prev_wo is not None:
                        emit_wo(*prev_wo)
                    prev_wo = (qt, attn_tiles)
                emit_wo(*prev_wo)
    return _legalize_single_wait(nc)


def host_prep(x, wq, wk, wv, wo, s=S, d=D, hq=HQ, ncores=NCORES):
    """Shared tensors + per-core weight shards, all host-side numpy."""
    scale = attn_scale(s, HD, MULT)
    xTf = np.ascontiguousarray(x.reshape(s, d).T)
    xT = xTf.astype(NPBF16)
    xT8 = xTf.astype(NPFP8)

    freq = ROPE_BASE ** (-(np.arange(0, HD, 2, dtype=np.float64) / HD))
    pos = np.arange(s, dtype=np.float64)
    angle = pos[:, None] * freq[None, :]  # [s, 64]
    cos = np.cos(angle).astype(NPBF16).T  # [64, s]
    sin = np.sin(angle).astype(NPBF16).T
    cosF = np.ascontiguousarray(np.concatenate([cos, cos], axis=0))
    sinSg = np.ascontiguousarray(np.concatenate([-sin, sin], axis=0))

    # triangular causal mask for diagonal chunks: keep iff p <= f
    p = np.arange(128)[:, None]
    f = np.arange(128)[None, :]
    maskT = (p <= f).astype(NPBF16)  # [128, 128]

    ident = np.eye(128, dtype=NPBF16)
    onesc = np.ones((128, 1), dtype=NPBF16)
    onesr = np.ones((128, 128), dtype=NPBF16)

    shared = dict(
        xT=xT, xT8=xT8, cosF=cosF, sinSg=sinSg, maskT=maskT, ident=ident,
        onesc=onesc, onesr=onesr,
    )

    in_maps = []
    for c in range(ncores):
        wq_c = wq[c * hq * 128 : (c + 1) * hq * 128, :]  # [hq*128, d]
        wk_c = wk[c * 128 : (c + 1) * 128, :]
        wv_c = wv[c * 128 : (c + 1) * 128, :] * scale
        wqk8 = np.ascontiguousarray(
            np.concatenate([wq_c.T, wk_c.T], axis=1)
        ).astype(NPFP8)  # [d, (hq+1)*128]
        wvT = np.ascontiguousarray(wv_c.T).astype(NPBF16)  # [d, 128]
        wo_c = wo[:, c * hq * 128 : (c + 1) * hq * 128]  # [d, hq*128]
        woT = np.ascontiguousarray(wo_c.T).astype(NPBF16)  # [hq*128, d]
        in_maps.append(dict(shared, wqk8=wqk8, wvT=wvT, woT=woT))
    return in_maps


_NC_CACHE = {}


def kernel(x, freqs_cis, wq, wk, wv, wo):
    del freqs_cis  # forward pass recomputes rope tables (matches reference)
    x = np.asarray(x, dtype=np.float32)
    key = (S, D, HQ)
    if key not in _NC_CACHE:
        _NC_CACHE[key] = build_core_kernel(S, D, HQ)
    nc = _NC_CACHE[key]
    in_maps = host_prep(
        x, np.asarray(wq, np.float32), np.asarray(wk, np.float32),
        np.asarray(wv, np.float32), np.asarray(wo, np.float32),
    )
    res = run_bass_kernel_spmd(nc, in_maps, core_ids=list(range(NCORES)))
    out = np.zeros((S, D), dtype=np.float32)
    for r in res.results:
        out += np.asarray(r["outp"], dtype=np.float32)
    return out.reshape(B, S, D)


if __name__ == "__main__":
    rng = np.random.default_rng(0)
    x = rng.standard_normal((B, S, D)).astype(np.float32)
    wq = (rng.standard_normal((H * HD, D)) * D**-0.5).astype(np.float32)
    wk = (rng.standard_normal((KVH * HD, D)) * D**-0.5).astype(np.float32)
    wv = (rng.standard_normal((KVH * HD, D)) * D**-0.5).astype(np.float32)
    wo = (rng.standard_normal((D, H * HD)) * (H * HD) ** -0.5).astype(np.float32)
    fc = rng.standard_normal((S, HD // 2)).astype(np.float32)
    out = kernel(x, fc, wq, wk, wv, wo)
    print(out.shape, out.dtype, np.abs(out).max())


# revision 25
# speedup vs baseline: 1.1426x; 1.0000x over previous
"""GQA attention layer (B=1, S=2048, D=4096, H=32, KVH=8, HD=128) on 8 TRN2
NeuronCores, tensor-parallel over heads.

Each core computes 4 query heads + their shared kv head end-to-end:
QKV projection -> RoPE -> causal attention (no-max-sub softmax, scores are
tiny) -> its slice of the wo projection. The 8 partial [S, D] outputs are
summed on the host (the "all-reduce after wo" of the sharding hint).

Device layouts (everything bf16 into the PE, fp32 PSUM accumulation):
  QT/KT  [HD=128(part), S]    from  lhsT=w[d,:], rhs=xT[d, s-tile]
  V      [S(part), HD]        via PE-transpose of VT
  scoresT[k(part), q]         lhsT=KT chunk, rhs=QT tile
  E = exp(scoresT/128) bf16; causal diagonal via 0/1 mask multiply
  attnT  [HD(part), q]        lhsT=V chunk, rhs=E  (accumulated over k)
  denom  [1, q]               lhsT=ones[128,1], rhs=E (accumulated over k)
  attnT_norm = attnT * bcast(1/denom)   (PE outer-product broadcast)
  out    [s(part), n]         lhsT=attnT_norm chunk, rhs=woT
"""

import json
import math

import ml_dtypes
import numpy as np

import concourse.bass as bass
import concourse.tile as tile
from concourse import mybir
from concourse.bass_utils import run_bass_kernel_spmd

BF16 = mybir.dt.bfloat16
F32 = mybir.dt.float32
FP8 = mybir.dt.float8e4
NPBF16 = ml_dtypes.bfloat16
NPFP8 = ml_dtypes.float8_e4m3

# Full problem constants
B, S, D = 1, 2048, 4096
H, KVH = 32, 8
HD = 128
NCORES = 8
HQ = H // NCORES  # query heads per core
MULT = 1.0
ROPE_BASE = 10000.0
ST = 512  # s-tile (PSUM bank width in fp32)


def attn_scale(seq_len=S, d_head=HD, mult=MULT):
    alpha = 1.0 / (1.0 + 4.0 * d_head / mult**2)
    lower = (math.log(seq_len) / seq_len) ** 0.5
    interp = math.exp((1.0 - alpha) * math.log(lower))
    return 1.0 / interp


def _legalize_single_wait(nc):
    """The walrus build in this container accepts only ONE sync wait per
    instruction ("Too many sync wait commands" in setupSyncWait). Split
    extra waits into preceding single-wait Drains (lowered to CTRL NOPs)
    on the same engine — same in-order stall semantics."""
    bir = json.loads(nc.to_json_bytes())
    ctr = 0
    for fn in bir["functions"]:
        for blk in fn["blocks"]:
            out = []
            for inst in blk["instructions"]:
                si = inst.get("sync_info")
                waits = (si or {}).get("on_wait") or []
                if len(waits) > 1:
                    for w in waits[:-1]:
                        ctr += 1
                        out.append(
                            {
                                "debug": inst.get("debug", 0),
                                "engine": inst["engine"],
                                "ins": [],
                                "name": f"{inst['name']}-mw{ctr}",
                                "opcode": "Drain",
                                "outs": [],
                                "sync_info": {"on_update": [], "on_wait": [w]},
                            }
                        )
                    si["on_wait"] = [waits[-1]]
                out.append(inst)
            blk["instructions"] = out
    fixed = json.dumps(bir).encode()
    nc.to_json_bytes = lambda: fixed
    return nc


def build_core_kernel(s=S, d=D, hq=HQ):
    """Bass module for one core: hq query heads + 1 kv head."""
    nst = s // ST  # s-tiles of 512
    ndk = d // 128  # contraction chunks
    nh = hq + 2  # q heads + k + v
    nnt = d // ST  # output n-tiles

    nqk = hq + 1  # q heads + k (fp8 path)

    nc = bass.Bass()
    xT_d = nc.dram_tensor("xT", [d, s], BF16, kind="ExternalInput")
    xT8_d = nc.dram_tensor("xT8", [d, s], FP8, kind="ExternalInput")
    wqk8_d = nc.dram_tensor("wqk8", [d, nqk * 128], FP8, kind="ExternalInput")
    wvT_d = nc.dram_tensor("wvT", [d, 128], BF16, kind="ExternalInput")
    woT_d = nc.dram_tensor("woT", [hq * 128, d], BF16, kind="ExternalInput")
    cosF_d = nc.dram_tensor("cosF", [128, s], BF16, kind="ExternalInput")
    sinSg_d = nc.dram_tensor("sinSg", [128, s], BF16, kind="ExternalInput")
    maskT_d = nc.dram_tensor("maskT", [128, 128], BF16, kind="ExternalInput")
    ident_d = nc.dram_tensor("ident", [128, 128], BF16, kind="ExternalInput")
    onesc_d = nc.dram_tensor("onesc", [128, 1], BF16, kind="ExternalInput")
    onesr_d = nc.dram_tensor("onesr", [128, 128], BF16, kind="ExternalInput")
    outp_d = nc.dram_tensor("outp", [s, d], BF16, kind="ExternalOutput")

    with tile.TileContext(nc) as tc:
        with (
            tc.tile_pool(name="const", bufs=1) as cp,
            tc.tile_pool(name="qkvsb", bufs=1) as qp,
            tc.tile_pool(name="xp", bufs=3) as xp,
            tc.tile_pool(name="rp", bufs=2) as rp,
            tc.tile_pool(name="vp", bufs=2) as vp,
            tc.tile_pool(name="ep", bufs=14) as ep,
            tc.tile_pool(name="sp", bufs=3) as sp,
            tc.tile_pool(name="op", bufs=8) as op,
            tc.tile_pool(name="at", bufs=8) as atp,
        ):
            # ---- resident constants ----
            # per-chunk weight tiles so the first matmul starts after the
            # first small DMA, not after the whole 10MB weight load
            npair = ndk // 2  # 256-row contraction pair-chunks (DoubleRow)
            w8g = [
                cp.tile([128, 4, 2, nqk * 128], FP8, tag=f"w8{g}", name=f"w8{g}")
                for g in range(npair // 4)
            ]
            wqk8_r = wqk8_d.rearrange("(g j ko p) m -> g p j ko m", j=4, ko=2, p=128)
            w8 = [w8g[j // 4][:, j % 4, :, :] for j in range(npair)]
            nwg = ndk // 8  # V weight groups of 8 contraction chunks
            wvsb4 = [
                cp.tile([128, 8, 128], BF16, tag=f"wv{g}", name=f"wv{g}")
                for g in range(nwg)
            ]
            wv_r = wvT_d.rearrange("(g c p) n -> g p c n", c=8, p=128)
            # interleave QK and V weight groups so quarter g's V matmuls
            # aren't stuck behind the whole fp8 weight load
            for g in range(npair // 4):
                nc.gpsimd.dma_start(w8g[g], wqk8_r[g])
                nc.gpsimd.dma_start(wvsb4[g], wv_r[g])
            wvsb = [wvsb4[dk // 8][:, dk % 8, :] for dk in range(ndk)]
            cossb = cp.tile([128, s], BF16)
            nc.gpsimd.dma_start(cossb, cosF_d[:])
            sinsb = cp.tile([128, s], BF16)
            nc.gpsimd.dma_start(sinsb, sinSg_d[:])
            masksb = cp.tile([128, 128], BF16)
            nc.gpsimd.dma_start(masksb, maskT_d[:])
            identsb = cp.tile([128, 128], BF16)
            nc.gpsimd.dma_start(identsb, ident_d[:])
            onescsb = cp.tile([128, 1], BF16)
            nc.gpsimd.dma_start(onescsb, onesc_d[:])
            onescbsb = cp.tile([128, 128], BF16)
            nc.gpsimd.dma_start(onescbsb, onesr_d[:])
            wosb = [
                cp.tile([128, d], BF16, tag=f"wo{mh}", name=f"wo{mh}")
                for mh in range(hq)
            ]
            for mh in range(hq):
                nc.gpsimd.dma_start(wosb[mh], woT_d[mh * 128 : (mh + 1) * 128, :])

            # ---- persistent activations (bf16) ----
            qt_sb = [
                qp.tile([128, s], BF16, tag=f"QT{h}", name=f"QT{h}")
                for h in range(hq)
            ]
            kt_sb = qp.tile([128, s], BF16, tag="KT")
            v_sb = qp.tile([128, s], BF16, tag="V")  # [s%128 part, (s//128)*HD]

            # ================= phase A: QKV projection + RoPE =================
            with (
                tc.tile_pool(name="psA", bufs=7, space="PSUM") as psA,
                tc.tile_pool(name="psT", bufs=1, space="PSUM") as psT,
            ):
                nq = 4  # quarters per s-tile
                ndkq = ndk // nq  # bf16 contraction chunks per quarter (V)
                npq = npair // nq  # fp8 pair-chunks per quarter (QK)
                for st in range(nst):
                    ssl = slice(st * ST, (st + 1) * ST)
                    acc = [
                        psA.tile([128, ST], F32, tag="acc", name=f"acc{h}")
                        for h in range(nh)
                    ]
                    # heads-major over resident xT quarters: at the next s-tile
                    # boundary only acc[0] must be free for PE to proceed
                    for quar in range(nq):
                        x8a = xp.tile([128, npq, 2, ST], FP8, tag="x8")
                        nc.sync.dma_start(
                            x8a,
                            xT8_d[quar * npq * 256 : (quar + 1) * npq * 256, ssl]
                            .rearrange("(i ko p) n -> p i ko n", ko=2, p=128),
                        )
                        xta = xp.tile([128, ndkq, ST], BF16, tag="xT")
                        nc.sync.dma_start(
                            xta,
                            xT_d[quar * ndkq * 128 : (quar + 1) * ndkq * 128, ssl]
                            .rearrange("(dk p) n -> p dk n", p=128),
                        )
                        # Q + K: fp8 DoubleRow, 256-deep contraction per matmul
                        for h in range(nqk):
                            for i in range(npq):
                                nc.tensor.matmul(
                                    acc[h],
                                    w8[quar * npq + i][:, :, h * 128 : (h + 1) * 128],
                                    x8a[:, i, :, :],
                                    start=(quar == 0 and i == 0),
                                    stop=(quar == nq - 1 and i == npq - 1),
                                    perf_mode=mybir.MatmulPerfMode.DoubleRow,
                                )
                        # V: bf16
                        for dk in range(ndkq):
                            nc.tensor.matmul(
                                acc[nh - 1],
                                wvsb[quar * ndkq + dk],
                                xta[:, dk, :],
                                start=(quar == 0 and dk == 0),
                                stop=(quar == nq - 1 and dk == ndkq - 1),
                            )
                    # RoPE for q heads and k; write bf16
                    for h in range(hq + 1):
                        dst = qt_sb[h] if h < hq else kt_sb
                        t1 = rp.tile([128, ST], F32, tag="t1")
                        nc.vector.tensor_mul(t1, acc[h], cossb[:, ssl])
                        tsw = rp.tile([128, ST], F32, tag="tsw")
                        nc.vector.tensor_copy(tsw[0:64, :], acc[h][64:128, :])
                        nc.vector.tensor_copy(tsw[64:128, :], acc[h][0:64, :])
                        nc.vector.tensor_mul(tsw, tsw, sinsb[:, ssl])
                        nc.vector.tensor_add(dst[:, ssl], t1, tsw)
                    # V: transpose [HD, s-tile] -> [s-chunk, HD] blocks
                    for j in range(ST // 128):
                        vtmp = vp.tile([128, 128], BF16, tag="vtmp")
                        nc.scalar.copy(vtmp, acc[hq + 1][:, j * 128 : (j + 1) * 128])
                        tp_ps = psT.tile([128, 128], BF16, tag="tp")
                        nc.tensor.transpose(tp_ps, vtmp, identsb)
                        sc = st * (ST // 128) + j
                        nc.vector.tensor_copy(
                            v_sb[:, sc * 128 : (sc + 1) * 128], tp_ps
                        )

            # ============ phase B: attention + output projection ============
            with (
                tc.tile_pool(name="psS", bufs=3, space="PSUM") as psS,
                tc.tile_pool(name="psD", bufs=1, space="PSUM") as psD,
                tc.tile_pool(name="psAt", bufs=4, space="PSUM") as psAt,
            ):

                def emit_wo(qt, attn_tiles):
                    # wo for the s-chunks of q-tile qt (emitted one q-tile
                    # late so the normalize tail overlaps the next q-tile's
                    # attention matmuls)
                    with nc.named_scope(f"wo{qt}"):
                        for j in range(ST // 128):
                            sc = qt * (ST // 128) + j
                            for nt in range(nnt):
                                o_ps = psS.tile(
                                    [128, ST], F32, tag="sc",
                                    name=f"wo{qt}_{j}_{nt}",
                                )
                                for mh in range(hq):
                                    nc.tensor.matmul(
                                        o_ps,
                                        attn_tiles[mh][:, j * 128 : (j + 1) * 128],
                                        wosb[mh][:, nt * ST : (nt + 1) * ST],
                                        start=(mh == 0),
                                        stop=(mh == hq - 1),
                                    )
                                osb = op.tile([128, ST], BF16, tag="osb")
                                nc.vector.tensor_copy(osb, o_ps)
                                nc.sync.dma_start(
                                    outp_d[
                                        sc * 128 : (sc + 1) * 128,
                                        nt * ST : (nt + 1) * ST,
                                    ],
                                    osb,
                                )

                prev_wo = None
                for qt in range(nst):
                    nk = (qt + 1) * (ST // 128)  # causal: k chunks this q-tile
                    attn_tiles = {}
                    with nc.named_scope(f"attn{qt}"):
                        # one denominator bank per q-tile: head h accumulates
                        # into partition row 32*h (distinct col-groups)
                        den4 = psD.tile([128, ST], F32, tag="den")
                        nc.vector.memset(den4, 1.0)
                        at_tiles = {
                            h: psAt.tile([128, ST], F32, tag="at", name=f"at{qt}_{h}")
                            for h in range(hq)
                        }
                        for c in range(nk):
                            # diagonal chunks: only columns >= 128*r valid
                            r = c - (nk - 4)
                            off = 128 * r if r > 0 else 0
                            w = ST - off
                            e_ts = {}
                            for h in range(hq):
                                sc_ps = psS.tile(
                                    [128, ST], F32, tag="sc", name=f"sc{qt}_{c}_{h}"
                                )
                                nc.tensor.matmul(
                                    sc_ps[:, 0:w],
                                    kt_sb[:, c * 128 : (c + 1) * 128],
                                    qt_sb[h][:, qt * ST + off : (qt + 1) * ST],
                                    start=True,
                                    stop=True,
                                )
                                e_t = ep.tile(
                                    [128, ST], BF16, tag="E", name=f"e{qt}_{c}_{h}"
                                )
                                nc.scalar.activation(
                                    e_t[:, 0:w],
                                    sc_ps[:, 0:w],
                                    mybir.ActivationFunctionType.Exp,
                                    scale=1.0 / HD,
                                )
                                if r >= 0:
                                    nc.vector.tensor_mul(
                                        e_t[:, 0:128], e_t[:, 0:128], masksb
                                    )
                                e_ts[h] = e_t
                            for h in range(hq):
                                nc.tensor.matmul(
                                    at_tiles[h][:, off:ST],
                                    v_sb[:, c * 128 : (c + 1) * 128],
                                    e_ts[h][:, 0:w],
                                    start=(c == 0),
                                    stop=(c == nk - 1),
                                )
                            # 4 single-row denominator matmuls in distinct
                            # col-groups: HW runs them concurrently
                            for h in range(hq):
                                nc.tensor.matmul(
                                    den4[32 * h : 32 * h + 1, off:ST],
                                    onescsb,
                                    e_ts[h][:, 0:w],
                                    start=(c == 0),
                                    stop=(c == nk - 1),
                                    tile_position=(0, 32 * h),
                                )
                        # one strided reciprocal for all 4 heads' denominators
                        recip = sp.tile([128, ST], F32, tag="recip", name=f"recip{qt}")
                        nc.vector.reciprocal(recip, den4)
                        recipb = sp.tile([128, ST], BF16, tag="recipb", name=f"recipb{qt}")
                        nc.scalar.copy(recipb, recip)
                        for hh in range(hq):
                            bc_ps = psS.tile(
                                [128, ST], F32, tag="sc", name=f"bc{qt}_{hh}"
                            )
                            nc.tensor.matmul(
                                bc_ps,
                                onescbsb[32 * hh : 32 * hh + 1, :],
                                recipb[32 * hh : 32 * hh + 1, :],
                                start=True,
                                stop=True,
                                tile_position=(32 * hh, 0),
                            )
                            bc_sb = sp.tile(
                                [128, ST], F32, tag="bcsb", name=f"bcsb{qt}_{hh}"
                            )
                            nc.scalar.copy(bc_sb, bc_ps)
                            atn = atp.tile([128, ST], BF16, tag="attnT")
                            nc.vector.tensor_mul(atn, at_tiles[hh], bc_sb)
                            attn_tiles[hh] = atn
                    if prev_wo is not None:
                        emit_wo(*prev_wo)
                    prev_wo = (qt, attn_tiles)
                emit_wo(*prev_wo)
    return _legalize_single_wait(nc)


def host_prep(x, wq, wk, wv, wo, s=S, d=D, hq=HQ, ncores=NCORES):
    """Shared tensors + per-core weight shards, all host-side numpy."""
    scale = attn_scale(s, HD, MULT)
    xTf = np.ascontiguousarray(x.reshape(s, d).T)
    xT = xTf.astype(NPBF16)
    xT8 = xTf.astype(NPFP8)

    freq = ROPE_BASE ** (-(np.arange(0, HD, 2, dtype=np.float64) / HD))
    pos = np.arange(s, dtype=np.float64)
    angle = pos[:, None] * freq[None, :]  # [s, 64]
    cos = np.cos(angle).astype(NPBF16).T  # [64, s]
    sin = np.sin(angle).astype(NPBF16).T
    cosF = np.ascontiguousarray(np.concatenate([cos, cos], axis=0))
    sinSg = np.ascontiguousarray(np.concatenate([-sin, sin], axis=0))

    # triangular causal mask for diagonal chunks: keep iff p <= f
    p = np.arange(128)[:, None]
    f = np.arange(128)[None, :]
    maskT = (p <= f).astype(NPBF16)  # [128, 128]

    ident = np.eye(128, dtype=NPBF16)
    onesc = np.ones((128, 1), dtype=NPBF16)
    onesr = np.ones((128, 128), dtype=NPBF16)

    shared = dict(
        xT=xT, xT8=xT8, cosF=cosF, sinSg=sinSg, maskT=maskT, ident=ident,
        onesc=onesc, onesr=onesr,
    )

    in_maps = []
    for c in range(ncores):
        wq_c = wq[c * hq * 128 : (c + 1) * hq * 128, :]  # [hq*128, d]
        wk_c = wk[c * 128 : (c + 1) * 128, :]
        wv_c = wv[c * 128 : (c + 1) * 128, :] * scale
        wqk8 = np.ascontiguousarray(
            np.concatenate([wq_c.T, wk_c.T], axis=1)
        ).astype(NPFP8)  # [d, (hq+1)*128]
        wvT = np.ascontiguousarray(wv_c.T).astype(NPBF16)  # [d, 128]
        wo_c = wo[:, c * hq * 128 : (c + 1) * hq * 128]  # [d, hq*128]
        woT = np.ascontiguousarray(wo_c.T).astype(NPBF16)  # [hq*128, d]
        in_maps.append(dict(shared, wqk8=wqk8, wvT=wvT, woT=woT))
    return in_maps


_NC_CACHE = {}


def kernel(x, freqs_cis, wq, wk, wv, wo):
    del freqs_cis  # forward pass recomputes rope tables (matches reference)
    x = np.asarray(x, dtype=np.float32)
    key = (S, D, HQ)
    if key not in _NC_CACHE:
        _NC_CACHE[key] = build_core_kernel(S, D, HQ)
    nc = _NC_CACHE[key]
    in_maps = host_prep(
        x, np.asarray(wq, np.float32), np.asarray(wk, np.float32),
        np.asarray(wv, np.float32), np.asarray(wo, np.float32),
    )
    res = run_bass_kernel_spmd(nc, in_maps, core_ids=list(range(NCORES)))
    out = np.zeros((S, D), dtype=np.float32)
    for r in res.results:
        out += np.asarray(r["outp"], dtype=np.float32)
    return out.reshape(B, S, D)


if __name__ == "__main__":
    rng = np.random.default_rng(0)
    x = rng.standard_normal((B, S, D)).astype(np.float32)
    wq = (rng.standard_normal((H * HD, D)) * D**-0.5).astype(np.float32)
    wk = (rng.standard_normal((KVH * HD, D)) * D**-0.5).astype(np.float32)
    wv = (rng.standard_normal((KVH * HD, D)) * D**-0.5).astype(np.float32)
    wo = (rng.standard_normal((D, H * HD)) * (H * HD) ** -0.5).astype(np.float32)
    fc = rng.standard_normal((S, HD // 2)).astype(np.float32)
    out = kernel(x, fc, wq, wk, wv, wo)
    print(out.shape, out.dtype, np.abs(out).max())
